# revision 23
# baseline (speedup 1.0000x reference)
"""Trainium2 Bass kernel for the GNN interaction layer (e3nn-style message passing).

Strategy: partition edges across 8 cores by receiver (2500 nodes/core), grouped
by 32-node receiver sub-blocks.  The spherical harmonics are folded into
host-precomputed bf16 "Y-scaled one-hot" matrices ohY[e, (l,m,n32)], streamed
from DRAM.  The scatter runs in swapped orientation on the PE: stationary =
per-edge gated features t_l (from the radial-MLP mix), moving = ohY, so the
per-node aggregate lands PSUM-transposed [c, (m,n)] and linear_down is a plain
per-irrep matmul with no transposes.  Two sub-blocks are processed concurrently
via tile_position column tiling.  Sender features are dma_gather'ed from a
device-computed h table; all gathers are issued up front so the Q7 descriptor
generation overlaps the main loop.  All matmuls are bf16.
"""
import math
import numpy as np
import ml_dtypes

from concourse import bacc, mybir, tile
from concourse.bass_utils import run_bass_kernel_spmd

F32 = mybir.dt.float32
BF16 = mybir.dt.bfloat16
I16 = mybir.dt.int16
AF = mybir.ActivationFunctionType
OP = mybir.AluOpType
BF = ml_dtypes.bfloat16

C = 64
R = 8
EPS = 0.5
N_NODES = 20000
N_EDGES = 320000
NCORES = 8
NS = N_NODES // NCORES          # nodes per core (2500)
SBN = 32                        # nodes per sub-block
NSB = 80                        # sub-blocks per core (79 real + 1 pad)
NPAIR = NSB // 2                # sub-block pairs (40)
# ohY column layout per chunk: l0 [0:32], l1 [32:128], l2 [128:288]
OHW = 288


def _spherical(v):
    u = v / np.linalg.norm(v, axis=-1, keepdims=True)
    x, y, z = u[:, 0], u[:, 1], u[:, 2]
    s15 = math.sqrt(15.0)
    y1 = math.sqrt(3.0) * u                                   # [E, 3]
    y2 = np.stack([
        s15 * x * y,
        s15 * y * z,
        0.5 * math.sqrt(5.0) * (3.0 * z * z - 1.0),
        s15 * x * z,
        0.5 * s15 * (x * x - y * y),
    ], axis=-1)                                               # [E, 5]
    return y1.astype(np.float32), y2.astype(np.float32)


def _host_prep(vectors, node_feats, radial, senders, receivers):
    senders = np.asarray(senders)
    receivers = np.asarray(receivers)
    vectors = np.asarray(vectors, np.float32)
    radial = np.asarray(radial, np.float32)

    core = receivers // NS
    rr = receivers % NS
    sb = rr // SBN                                # 0..78
    n32 = rr % SBN
    gkey = core * NSB + sb
    order = np.argsort(gkey, kind="stable")
    counts = np.bincount(gkey, minlength=NCORES * NSB)
    CH = max(2, int(math.ceil(counts.max() / 128.0)))
    SBW = CH * 128                                # slots per sub-block
    TOT = NSB * SBW                               # padded slots per core
    NCH = TOT // 128
    NG = TOT // 512
    NPG = NG // 2
    NCALL = TOT // 1024
    assert TOT % 1024 == 0

    # slot index for each edge (in sorted order)
    starts = np.concatenate([[0], np.cumsum(counts)])
    rank = np.arange(len(order)) - starts[gkey[order]]
    g_ord = gkey[order]
    slot = (g_ord % NSB) * SBW + rank             # slot within its core
    core_ord = g_ord // NSB

    y1, y2 = _spherical(vectors)
    y1o, y2o = y1[order], y2[order]
    n32o = n32[order]
    sndo = senders[order].astype(np.int16)
    rado = radial[order]

    snd = np.zeros((NCORES, TOT), np.int16)
    rad = np.zeros((NCORES, TOT, R), np.float32)
    snd[core_ord, slot] = sndo
    rad[core_ord, slot] = rado

    # ohY: [NCORES, TOT, 288] fp32 -> consumption-ordered bf16
    # one row per pair: [128, CH*2*OHW] (j-major, halves side by side)
    ohY_d = np.zeros((NCORES, NPAIR, 128, CH * 2 * OHW), BF)
    for k in range(NCORES):
        m = core_ord == k
        sl = slot[m]
        oh = np.zeros((TOT, OHW), np.float32)
        rows = sl
        oh[rows, n32o[m]] = 1.0
        for mm in range(3):
            oh[rows, 32 + 32 * mm + n32o[m]] = y1o[m, mm]
        for mm in range(5):
            oh[rows, 128 + 32 * mm + n32o[m]] = y2o[m, mm]
        # pair packing: per step j the 576 cols are l-grouped [A_l | B_l]:
        # l0: A32|B32, l1: A96|B96, l2: A160|B160
        ohp = oh.reshape(NPAIR, 2, CH, 128, OHW)
        dst = np.empty((NPAIR, 128, CH, 2 * OHW), np.float32)
        for l, (c0, c1) in enumerate(((0, 32), (32, 128), (128, OHW))):
            w = c1 - c0
            o0 = 2 * c0
            dst[:, :, :, o0:o0 + w] = ohp[:, 0, :, :, c0:c1].transpose(0, 2, 1, 3)
            dst[:, :, :, o0 + w:o0 + 2 * w] = \
                ohp[:, 1, :, :, c0:c1].transpose(0, 2, 1, 3)
        ohY_d[k] = dst.reshape(NPAIR, 128, CH * 2 * OHW).astype(BF)

    # gather idx: wrapped [16, 64] per 1024-slot call, tiled to 128 partitions
    idx = np.zeros((NCORES, 128, NCALL * 64), np.int16)
    for cidx in range(NCALL):
        blk = snd[:, cidx * 1024:(cidx + 1) * 1024]          # [NCORES, 1024]
        wrapped = blk.reshape(NCORES, 64, 16).transpose(0, 2, 1)
        idx[:, :, cidx * 64:(cidx + 1) * 64] = np.tile(wrapped, (1, 8, 1))

    # radial, transposed, packed per MLP pair: rows 0:8 even group, 8:16 odd
    radT = np.zeros((NCORES, 16, NPG * 512), BF)
    radt = rad.transpose(0, 2, 1)                            # [NCORES, R, TOT]
    for pg in range(NPG):
        radT[:, 0:8, pg * 512:(pg + 1) * 512] = \
            radt[:, :, (2 * pg) * 512:(2 * pg + 1) * 512]
        radT[:, 8:16, pg * 512:(pg + 1) * 512] = \
            radt[:, :, (2 * pg + 1) * 512:(2 * pg + 2) * 512]

    return dict(CH=CH, TOT=TOT, NCH=NCH, NG=NG, NPG=NPG, NCALL=NCALL,
                idx=idx, ohY=ohY_d, radT=radT)


def _scaled_weights(w_up, w1, w2, w3, w4, wd0, wd1, wd2):
    inv_sqrt_c = 1.0 / math.sqrt(C)
    w1s = (np.asarray(w1) / math.sqrt(R)).astype(np.float32)
    w2s = (np.asarray(w2) / math.sqrt(64.0)).astype(np.float32)
    w3s = (np.asarray(w3) / math.sqrt(64.0)).astype(np.float32)
    w4s = (np.asarray(w4) * (1.0 / math.sqrt(64.0)) * (1.0 / C)).astype(np.float32)
    w1d = np.zeros((128, 64), np.float32)
    w1d[0:R] = w1s
    w1d[64:64 + R] = w1s
    w2d = np.concatenate([w2s, w2s], axis=0)
    w3d = np.concatenate([w3s, w3s], axis=0)
    w4d = np.concatenate([w4s, w4s], axis=0)
    # block-diagonal per irrep: one matmul handles both halves
    wdd = np.zeros((128, 3, 128), np.float32)
    for i, wd in enumerate((wd0, wd1, wd2)):
        s = np.asarray(wd) * EPS * inv_sqrt_c
        wdd[0:64, i, 0:64] = s
        wdd[64:128, i, 64:128] = s
    return dict(
        wup=(np.asarray(w_up) * inv_sqrt_c).astype(BF),
        w1d=w1d.astype(BF), w2d=w2d.astype(BF), w3d=w3d.astype(BF),
        w4d=w4d.astype(BF), wdd=wdd.astype(BF),
    )


def _emit_mlp_pair(nc, apool, psm, pg, radT_d, w1d, w2d, w3d):
    """MLP layers 1-3 for groups 2*pg (partitions 0-63) and 2*pg+1 (64-127)."""
    rt = apool.tile([128, 512], BF16, tag="radT")
    nc.sync.dma_start(out=rt[0:R], in_=radT_d[0:8, pg * 512:(pg + 1) * 512])
    nc.scalar.dma_start(out=rt[64:64 + R],
                        in_=radT_d[8:16, pg * 512:(pg + 1) * 512])

    ps1 = psm.tile([128, 512], F32, tag="mlp")
    nc.tensor.matmul(ps1[0:64], w1d[0:R], rt[0:R], start=True, stop=True,
                     tile_position=(0, 0))
    nc.tensor.matmul(ps1[64:128], w1d[64:64 + R], rt[64:64 + R], start=True,
                     stop=True, tile_position=(64, 64))
    a1 = apool.tile([128, 512], BF16, tag="a1")
    nc.scalar.activation(a1[:], ps1[:], AF.Silu)

    ps2 = psm.tile([128, 512], F32, tag="mlp")
    nc.tensor.matmul(ps2[0:64], w2d[0:64], a1[0:64], start=True, stop=True,
                     tile_position=(0, 0))
    nc.tensor.matmul(ps2[64:128], w2d[64:128], a1[64:128], start=True,
                     stop=True, tile_position=(64, 64))
    a2 = apool.tile([128, 512], BF16, tag="a2")
    nc.scalar.activation(a2[:], ps2[:], AF.Silu)

    ps3 = psm.tile([128, 512], F32, tag="mlp")
    nc.tensor.matmul(ps3[0:64], w3d[0:64], a2[0:64], start=True, stop=True,
                     tile_position=(0, 0))
    nc.tensor.matmul(ps3[64:128], w3d[64:128], a2[64:128], start=True,
                     stop=True, tile_position=(64, 64))
    a3 = apool.tile([128, 512], BF16, tag="a3")
    nc.scalar.activation(a3[:], ps3[:], AF.Silu)
    return a3


def _build(CH):
    SBW = CH * 128
    TOT = NSB * SBW
    NCH = TOT // 128
    NG = TOT // 512
    NPG = NG // 2
    NCALL = TOT // 1024

    nc = bacc.Bacc(None, target_bir_lowering=False, debug=False,
                   dynamic_dma_scratch_size=16384, num_swdge_queues=2)

    nfT_d = nc.dram_tensor("nfT", [C, N_NODES], BF16, kind="ExternalInput")
    wup_d = nc.dram_tensor("wup", [C, C], BF16, kind="ExternalInput")
    w1_d = nc.dram_tensor("w1d", [128, 64], BF16, kind="ExternalInput")
    w2_d = nc.dram_tensor("w2d", [128, 64], BF16, kind="ExternalInput")
    w3_d = nc.dram_tensor("w3d", [128, 64], BF16, kind="ExternalInput")
    w4_d = nc.dram_tensor("w4d", [128, 3 * C], BF16, kind="ExternalInput")
    wdd_d = nc.dram_tensor("wdd", [128, 3, 128], BF16, kind="ExternalInput")
    idx_d = nc.dram_tensor("idx", [128, NCALL * 64], I16, kind="ExternalInput")
    ohY_d = nc.dram_tensor("ohY", [NPAIR, 128, CH * 2 * OHW], BF16,
                           kind="ExternalInput")
    radT_d = nc.dram_tensor("radT", [16, NPG * 512], BF16, kind="ExternalInput")

    h_d = nc.dram_tensor("h", [N_NODES, C], F32)
    out_d = nc.dram_tensor("outp", [NPAIR, 128, OHW], F32, kind="ExternalOutput")

    with tile.TileContext(nc) as tc:
        with tc.tile_pool(name="const", bufs=1) as cpool:
            wup = cpool.tile([C, C], BF16)
            nc.sync.dma_start(out=wup[:], in_=wup_d[:])
            w1d = cpool.tile([128, 64], BF16, tag="w1d")
            nc.sync.dma_start(out=w1d[:], in_=w1_d[:])
            w2d = cpool.tile([128, 64], BF16, tag="w2d")
            nc.sync.dma_start(out=w2d[:], in_=w2_d[:])
            w3d = cpool.tile([128, 64], BF16, tag="w3d")
            nc.sync.dma_start(out=w3d[:], in_=w3_d[:])
            w4d = cpool.tile([128, 3 * C], BF16, tag="w4d")
            nc.sync.dma_start(out=w4d[:], in_=w4_d[:])
            wdd = cpool.tile([128, 3, 128], BF16, tag="wdd")
            nc.sync.dma_start(out=wdd[:], in_=wdd_d[:])
            idxt = cpool.tile([128, NCALL * 64], I16)
            nc.sync.dma_start(out=idxt[:], in_=idx_d[:])

            # all gathered sender features stay resident in SBUF
            sres = cpool.tile([128, NCH, C], F32, tag="sres")

            with tc.tile_pool(name="ohp", bufs=2) as ohp, \
                 tc.tile_pool(name="ap", bufs=4) as apool, \
                 tc.tile_pool(name="tp", bufs=6) as tpool, \
                 tc.tile_pool(name="wr", bufs=2) as wrp, \
                 tc.tile_pool(name="psm", bufs=1, space="PSUM") as psm, \
                 tc.tile_pool(name="psx", bufs=2, space="PSUM") as psx, \
                 tc.tile_pool(name="psa", bufs=2, space="PSUM") as psa, \
                 tc.tile_pool(name="pso", bufs=1, space="PSUM") as pso:

                # ---- phase 1: h = nf @ wup (nfT streamed per batch) ----
                with tc.tile_pool(name="hsb", bufs=3) as hsb:
                    NFULL = N_NODES // 128                    # 156 full tiles
                    for b in range(0, NFULL, 8):
                        nt = min(8, NFULL - b)
                        nft = hsb.tile([C, 8 * 128], BF16, tag="nft")
                        nc.sync.dma_start(
                            out=nft[:, :nt * 128],
                            in_=nfT_d[:, b * 128:(b + nt) * 128])
                        hb = hsb.tile([128, 8, C], F32, tag="hsb")
                        for t0 in range(0, nt, 4):
                            tn = min(4, nt - t0)
                            ps = psm.tile([128, 4, C], F32, tag="mlp")
                            for t in range(t0, t0 + tn):
                                nc.tensor.matmul(
                                    ps[:, t - t0, :],
                                    nft[:, t * 128:(t + 1) * 128],
                                    wup[:], start=(t == t0), stop=True,
                                    skip_group_check=True)
                            nc.vector.tensor_copy(hb[:, t0:t0 + tn, :],
                                                  ps[:, :tn, :])
                        nc.scalar.dma_start(
                            out=h_d[b * 128:(b + nt) * 128].rearrange(
                                "(t p) c -> p t c", p=128),
                            in_=hb[:, :nt, :])
                    rem = N_NODES - NFULL * 128               # 32 tail rows
                    if rem:
                        nft = hsb.tile([C, 8 * 128], BF16, tag="nft")
                        nc.sync.dma_start(out=nft[:, :rem],
                                          in_=nfT_d[:, NFULL * 128:])
                        ps = psm.tile([128, 4, C], F32, tag="mlp")
                        nc.tensor.matmul(ps[:rem, 0, :], nft[:, :rem],
                                         wup[:], start=True, stop=True)
                        hb = hsb.tile([128, 8, C], F32, tag="hsb")
                        nc.vector.tensor_copy(hb[:rem, 0, :], ps[:rem, 0, :])
                        nc.scalar.dma_start(out=h_d[NFULL * 128:],
                                            in_=hb[:rem, 0, :])

                # ---- prologue: issue all gathers (Q7 gen overlaps main) ----
                for cidx in range(NCALL):
                    nc.gpsimd.dma_gather(
                        sres[:, cidx * 8:(cidx + 1) * 8, :], h_d[:],
                        idxt[:, cidx * 64:(cidx + 1) * 64],
                        1024, 1024, C, queue_num=cidx % 2)

                # ---- main loop over sub-block pairs ----
                # acc layout (double-width, garbage quadrants unused):
                #   acc1 [128, 256]: l0 [0:64] (A=[0:64,0:32], B=[64:128,32:64])
                #                    l1 [64:256] (A=[0:64,64:160], B=[64:,160:])
                #   acc2 [128, 320]: l2 (A=[0:64,0:160], B=[64:128,160:320])
                LW = ((0, 32), (32, 128), (128, OHW))
                next_pg = 0
                a3t = {}
                for p in range(NPAIR):
                    acc1 = psa.tile([128, 256], F32, tag="acc1")
                    acc2 = psa.tile([128, 320], F32, tag="acc2")
                    ohtp = ohp.tile([128, CH, 2 * OHW], BF16, tag="ohY")
                    nc.sync.dma_start(out=ohtp[:], in_=ohY_d[p])
                    for j in range(CH):
                        # t stationary for both halves: [128, (l, half, c)]
                        tt = tpool.tile([128, 3, 2, C], BF16, tag="t_all")
                        mix = psx.tile([128, 2, 3 * C], F32, tag="mix")
                        for half in range(2):
                            ch = (2 * p + half) * CH + j
                            G, sub = divmod(ch, 4)
                            pg, parity = divmod(G, 2)
                            while next_pg <= pg and next_pg < NPG:
                                a3t[next_pg] = _emit_mlp_pair(
                                    nc, apool, psm, next_pg, radT_d,
                                    w1d, w2d, w3d)
                                next_pg += 1
                            p0 = 64 * parity
                            nc.tensor.matmul(
                                mix[:, half, :],
                                a3t[pg][p0:p0 + 64, sub * 128:(sub + 1) * 128],
                                w4d[p0:p0 + 64], start=(half == 0), stop=True,
                                tile_position=(p0, 0),
                                skip_group_check=True)
                            mixv = mix[:, half, :].rearrange(
                                "p (i c) -> p i c", i=3)
                            sv = sres[:, ch, :].unsqueeze(1) \
                                .broadcast_to((128, 3, C))
                            nc.vector.tensor_tensor(tt[:, :, half, :], mixv,
                                                    sv, OP.mult)

                        # 3 scatter matmuls; moving = [ohY_A_l | ohY_B_l]
                        sp = j == CH - 1
                        nc.tensor.matmul(
                            acc1[:, 0:64], tt[:, 0, :, :],
                            ohtp[:, j, 0:64],
                            start=(j == 0), stop=sp, skip_group_check=True)
                        nc.tensor.matmul(
                            acc1[:, 64:256], tt[:, 1, :, :],
                            ohtp[:, j, 64:256],
                            start=False, stop=sp, skip_group_check=True)
                        nc.tensor.matmul(
                            acc2[:], tt[:, 2, :, :],
                            ohtp[:, j, 256:576],
                            start=(j == 0), stop=sp, skip_group_check=True)

                    # ---- pair wrap-up: linear_down + output ----
                    # collect valid quadrants into aggs [128, (l0,l1,l2)]
                    aggs = wrp.tile([128, OHW], BF16, tag="aggs")
                    srcs = [(acc1, 0, 0, 32), (acc1, 64, 32, 128),
                            (acc2, 0, 128, OHW)]
                    for l, (accs, a0, c0, c1) in enumerate(srcs):
                        w = c1 - c0
                        nc.scalar.copy(aggs[0:64, c0:c1],
                                       accs[0:64, a0:a0 + w])
                        nc.vector.tensor_copy(aggs[64:128, c0:c1],
                                              accs[64:128, a0 + w:a0 + 2 * w])
                    o = pso.tile([128, OHW], F32, tag="o")
                    for l, (c0, c1) in enumerate(LW):
                        nc.tensor.matmul(
                            o[:, c0:c1], wdd[:, l, :], aggs[:, c0:c1],
                            start=(l == 0), stop=True, skip_group_check=True)
                    osb = wrp.tile([128, OHW], F32, tag="osb")
                    nc.scalar.copy(osb[:], o[:])
                    nc.sync.dma_start(out=out_d[p], in_=osb[:])

    nc.compile()
    return nc


_CACHE = {}


def _get_program(CH):
    if CH not in _CACHE:
        _CACHE[CH] = _build(CH)
    return _CACHE[CH]


def _make_in_maps(prep, sw, node_feats):
    nfT = np.ascontiguousarray(np.asarray(node_feats, np.float32).T).astype(BF)
    maps = []
    for k in range(NCORES):
        maps.append({
            "nfT": nfT, "wup": sw["wup"], "w1d": sw["w1d"], "w2d": sw["w2d"],
            "w3d": sw["w3d"], "w4d": sw["w4d"], "wdd": sw["wdd"],
            "idx": prep["idx"][k], "ohY": prep["ohY"][k],
            "radT": prep["radT"][k],
        })
    return maps


def _assemble(results):
    out = np.empty((N_NODES, 9 * C), np.float32)
    for k in range(NCORES):
        oc = results[k]["outp"]                     # [NPAIR, 128, 288]
        # -> [NSB=80, 64, 288]
        ocs = oc.reshape(NPAIR, 2, 64, OHW).reshape(NSB, 64, OHW)
        nsb_real = (NS + SBN - 1) // SBN            # 79
        for s in range(nsb_real):
            nn = min(SBN, NS - s * SBN)
            M = ocs[s]                              # [64 d, 288]
            r0 = k * NS + s * SBN
            out[r0:r0 + nn, 0:C] = M[:, 0:nn].T
            m1 = M[:, 32:128].reshape(64, 3, SBN)   # [d, m, n]
            out[r0:r0 + nn, C:4 * C] = m1[:, :, :nn].transpose(2, 0, 1) \
                .reshape(nn, 3 * C)
            m2 = M[:, 128:OHW].reshape(64, 5, SBN)
            out[r0:r0 + nn, 4 * C:] = m2[:, :, :nn].transpose(2, 0, 1) \
                .reshape(nn, 5 * C)
    return out


def kernel(vectors, node_feats, radial_embedding, senders, receivers,
           w_up, mlp_w1, mlp_w2, mlp_w3, mlp_w4,
           w_down0, w_down1, w_down2):
    prep = _host_prep(vectors, node_feats, radial_embedding, senders, receivers)
    sw = _scaled_weights(w_up, mlp_w1, mlp_w2, mlp_w3, mlp_w4,
                         w_down0, w_down1, w_down2)
    nc = _get_program(prep["CH"])
    in_maps = _make_in_maps(prep, sw, node_feats)
    res = run_bass_kernel_spmd(nc, in_maps, list(range(NCORES)))
    return _assemble(res.results)


# revision 26
# speedup vs baseline: 1.5607x; 1.5607x over previous
"""Trainium2 Bass kernel for the GNN interaction layer (e3nn-style message passing).

Strategy: partition edges across 8 cores by receiver (2500 nodes/core), grouped
by 32-node receiver sub-blocks.  The spherical harmonics are folded into
host-precomputed bf16 "Y-scaled one-hot" matrices ohY[e, (l,m,n32)], streamed
from DRAM.  The scatter runs in swapped orientation on the PE: stationary =
per-edge gated features t_l (from the radial-MLP mix), moving = ohY, so the
per-node aggregate lands PSUM-transposed [c, (m,n)] and linear_down is a plain
per-irrep matmul with no transposes.  Two sub-blocks are processed concurrently
via tile_position column tiling.  Sender features are dma_gather'ed from a
device-computed h table; all gathers are issued up front so the Q7 descriptor
generation overlaps the main loop.  All matmuls are bf16.
"""
import math
import numpy as np
import ml_dtypes

from concourse import bacc, mybir, tile
from concourse.bass_utils import run_bass_kernel_spmd

F32 = mybir.dt.float32
BF16 = mybir.dt.bfloat16
I16 = mybir.dt.int16
AF = mybir.ActivationFunctionType
OP = mybir.AluOpType
BF = ml_dtypes.bfloat16

C = 64
R = 8
EPS = 0.5
N_NODES = 20000
N_EDGES = 320000
NCORES = 8
NS = N_NODES // NCORES          # nodes per core (2500)
SBN = 32                        # nodes per sub-block
NSB = 80                        # sub-blocks per core (79 real + 1 pad)
NPAIR = NSB // 2                # sub-block pairs (40)
# ohY column layout per chunk: l0 [0:32], l1 [32:128], l2 [128:288]
OHW = 288


def _spherical(v):
    u = v / np.linalg.norm(v, axis=-1, keepdims=True)
    x, y, z = u[:, 0], u[:, 1], u[:, 2]
    s15 = math.sqrt(15.0)
    y1 = math.sqrt(3.0) * u                                   # [E, 3]
    y2 = np.stack([
        s15 * x * y,
        s15 * y * z,
        0.5 * math.sqrt(5.0) * (3.0 * z * z - 1.0),
        s15 * x * z,
        0.5 * s15 * (x * x - y * y),
    ], axis=-1)                                               # [E, 5]
    return y1.astype(np.float32), y2.astype(np.float32)


def _host_prep(vectors, node_feats, radial, senders, receivers):
    senders = np.asarray(senders)
    receivers = np.asarray(receivers)
    vectors = np.asarray(vectors, np.float32)
    radial = np.asarray(radial, np.float32)

    core = receivers // NS
    rr = receivers % NS
    sb = rr // SBN                                # 0..78
    n32 = rr % SBN
    gkey = core * NSB + sb
    order = np.argsort(gkey, kind="stable")
    counts = np.bincount(gkey, minlength=NCORES * NSB)
    CH = max(2, int(math.ceil(counts.max() / 128.0)))
    SBW = CH * 128                                # slots per sub-block
    TOT = NSB * SBW                               # padded slots per core
    NCH = TOT // 128
    NG = TOT // 512
    NPG = NG // 2
    NCALL = TOT // 1024
    assert TOT % 1024 == 0

    # slot index for each edge (in sorted order)
    starts = np.concatenate([[0], np.cumsum(counts)])
    rank = np.arange(len(order)) - starts[gkey[order]]
    g_ord = gkey[order]
    slot = (g_ord % NSB) * SBW + rank             # slot within its core
    core_ord = g_ord // NSB

    y1, y2 = _spherical(vectors)
    y1o, y2o = y1[order], y2[order]
    n32o = n32[order]
    sndo = senders[order].astype(np.int16)
    rado = radial[order]

    snd = np.zeros((NCORES, TOT), np.int16)
    rad = np.zeros((NCORES, TOT, R), np.float32)
    snd[core_ord, slot] = sndo
    rad[core_ord, slot] = rado

    # ohY: [NCORES, TOT, 288] fp32 -> consumption-ordered bf16
    # one row per pair: [128, CH*2*OHW] (j-major, halves side by side)
    ohY_d = np.zeros((NCORES, NPAIR, 128, CH * 2 * OHW), BF)
    for k in range(NCORES):
        m = core_ord == k
        sl = slot[m]
        oh = np.zeros((TOT, OHW), np.float32)
        rows = sl
        oh[rows, n32o[m]] = 1.0
        for mm in range(3):
            oh[rows, 32 + 32 * mm + n32o[m]] = y1o[m, mm]
        for mm in range(5):
            oh[rows, 128 + 32 * mm + n32o[m]] = y2o[m, mm]
        # pair packing: per step j the 576 cols are l-grouped [A_l | B_l]:
        # l0: A32|B32, l1: A96|B96, l2: A160|B160
        ohp = oh.reshape(NPAIR, 2, CH, 128, OHW)
        dst = np.empty((NPAIR, 128, CH, 2 * OHW), np.float32)
        for l, (c0, c1) in enumerate(((0, 32), (32, 128), (128, OHW))):
            w = c1 - c0
            o0 = 2 * c0
            dst[:, :, :, o0:o0 + w] = ohp[:, 0, :, :, c0:c1].transpose(0, 2, 1, 3)
            dst[:, :, :, o0 + w:o0 + 2 * w] = \
                ohp[:, 1, :, :, c0:c1].transpose(0, 2, 1, 3)
        ohY_d[k] = dst.reshape(NPAIR, 128, CH * 2 * OHW).astype(BF)

    # gather idx: wrapped [16, 64] per 1024-slot call, tiled to 128 partitions
    idx = np.zeros((NCORES, 128, NCALL * 64), np.int16)
    for cidx in range(NCALL):
        blk = snd[:, cidx * 1024:(cidx + 1) * 1024]          # [NCORES, 1024]
        wrapped = blk.reshape(NCORES, 64, 16).transpose(0, 2, 1)
        idx[:, :, cidx * 64:(cidx + 1) * 64] = np.tile(wrapped, (1, 8, 1))

    # radial, transposed, packed per MLP pair: rows 0:8 even group, 8:16 odd
    radT = np.zeros((NCORES, 16, NPG * 512), BF)
    radt = rad.transpose(0, 2, 1)                            # [NCORES, R, TOT]
    for pg in range(NPG):
        radT[:, 0:8, pg * 512:(pg + 1) * 512] = \
            radt[:, :, (2 * pg) * 512:(2 * pg + 1) * 512]
        radT[:, 8:16, pg * 512:(pg + 1) * 512] = \
            radt[:, :, (2 * pg + 1) * 512:(2 * pg + 2) * 512]

    return dict(CH=CH, TOT=TOT, NCH=NCH, NG=NG, NPG=NPG, NCALL=NCALL,
                idx=idx, ohY=ohY_d, radT=radT)


def _scaled_weights(w_up, w1, w2, w3, w4, wd0, wd1, wd2):
    inv_sqrt_c = 1.0 / math.sqrt(C)
    w1s = (np.asarray(w1) / math.sqrt(R)).astype(np.float32)
    w2s = (np.asarray(w2) / math.sqrt(64.0)).astype(np.float32)
    w3s = (np.asarray(w3) / math.sqrt(64.0)).astype(np.float32)
    w4s = (np.asarray(w4) * (1.0 / math.sqrt(64.0)) * (1.0 / C)).astype(np.float32)
    w1d = np.zeros((128, 64), np.float32)
    w1d[0:R] = w1s
    w1d[64:64 + R] = w1s
    w2d = np.concatenate([w2s, w2s], axis=0)
    w3d = np.concatenate([w3s, w3s], axis=0)
    w4d = np.concatenate([w4s, w4s], axis=0)
    # block-diagonal per irrep: one matmul handles both halves
    wdd = np.zeros((128, 3, 128), np.float32)
    for i, wd in enumerate((wd0, wd1, wd2)):
        s = np.asarray(wd) * EPS * inv_sqrt_c
        wdd[0:64, i, 0:64] = s
        wdd[64:128, i, 64:128] = s
    return dict(
        wup=(np.asarray(w_up) * inv_sqrt_c).astype(BF),
        w1d=w1d.astype(BF), w2d=w2d.astype(BF), w3d=w3d.astype(BF),
        w4d=w4d.astype(BF), wdd=wdd.astype(BF),
    )


def _emit_mlp_pair(nc, apool, psm, pg, radT_d, w1d, w2d, w3d):
    """MLP layers 1-3 for groups 2*pg (partitions 0-63) and 2*pg+1 (64-127)."""
    rt = apool.tile([128, 512], BF16, tag="radT")
    nc.sync.dma_start(out=rt[0:R], in_=radT_d[0:8, pg * 512:(pg + 1) * 512])
    nc.scalar.dma_start(out=rt[64:64 + R],
                        in_=radT_d[8:16, pg * 512:(pg + 1) * 512])

    ps1 = psm.tile([128, 512], F32, tag="mlp")
    nc.tensor.matmul(ps1[0:64], w1d[0:R], rt[0:R], start=True, stop=True,
                     tile_position=(0, 0))
    nc.tensor.matmul(ps1[64:128], w1d[64:64 + R], rt[64:64 + R], start=True,
                     stop=True, tile_position=(64, 64))
    a1 = apool.tile([128, 512], BF16, tag="a1")
    nc.scalar.activation(a1[:], ps1[:], AF.Silu)

    ps2 = psm.tile([128, 512], F32, tag="mlp")
    nc.tensor.matmul(ps2[0:64], w2d[0:64], a1[0:64], start=True, stop=True,
                     tile_position=(0, 0))
    nc.tensor.matmul(ps2[64:128], w2d[64:128], a1[64:128], start=True,
                     stop=True, tile_position=(64, 64))
    a2 = apool.tile([128, 512], BF16, tag="a2")
    nc.scalar.activation(a2[:], ps2[:], AF.Silu)

    ps3 = psm.tile([128, 512], F32, tag="mlp")
    nc.tensor.matmul(ps3[0:64], w3d[0:64], a2[0:64], start=True, stop=True,
                     tile_position=(0, 0))
    nc.tensor.matmul(ps3[64:128], w3d[64:128], a2[64:128], start=True,
                     stop=True, tile_position=(64, 64))
    a3 = apool.tile([128, 512], BF16, tag="a3")
    nc.scalar.activation(a3[:], ps3[:], AF.Silu)
    return a3


def _build(CH):
    SBW = CH * 128
    TOT = NSB * SBW
    NCH = TOT // 128
    NG = TOT // 512
    NPG = NG // 2
    NCALL = TOT // 1024

    nc = bacc.Bacc(None, target_bir_lowering=False, debug=False,
                   dynamic_dma_scratch_size=16384, num_swdge_queues=2)

    nfT_d = nc.dram_tensor("nfT", [C, N_NODES], BF16, kind="ExternalInput")
    wup_d = nc.dram_tensor("wup", [C, C], BF16, kind="ExternalInput")
    w1_d = nc.dram_tensor("w1d", [128, 64], BF16, kind="ExternalInput")
    w2_d = nc.dram_tensor("w2d", [128, 64], BF16, kind="ExternalInput")
    w3_d = nc.dram_tensor("w3d", [128, 64], BF16, kind="ExternalInput")
    w4_d = nc.dram_tensor("w4d", [128, 3 * C], BF16, kind="ExternalInput")
    wdd_d = nc.dram_tensor("wdd", [128, 3, 128], BF16, kind="ExternalInput")
    idx_d = nc.dram_tensor("idx", [128, NCALL * 64], I16, kind="ExternalInput")
    ohY_d = nc.dram_tensor("ohY", [NPAIR, 128, CH * 2 * OHW], BF16,
                           kind="ExternalInput")
    radT_d = nc.dram_tensor("radT", [16, NPG * 512], BF16, kind="ExternalInput")

    h_d = nc.dram_tensor("h", [N_NODES, C], F32)
    out_d = nc.dram_tensor("outp", [NPAIR, 128, OHW], F32, kind="ExternalOutput")

    with tile.TileContext(nc) as tc:
        with tc.tile_pool(name="const", bufs=1) as cpool:
            wup = cpool.tile([C, C], BF16)
            nc.sync.dma_start(out=wup[:], in_=wup_d[:])
            w1d = cpool.tile([128, 64], BF16, tag="w1d")
            nc.sync.dma_start(out=w1d[:], in_=w1_d[:])
            w2d = cpool.tile([128, 64], BF16, tag="w2d")
            nc.sync.dma_start(out=w2d[:], in_=w2_d[:])
            w3d = cpool.tile([128, 64], BF16, tag="w3d")
            nc.sync.dma_start(out=w3d[:], in_=w3_d[:])
            w4d = cpool.tile([128, 3 * C], BF16, tag="w4d")
            nc.sync.dma_start(out=w4d[:], in_=w4_d[:])
            wdd = cpool.tile([128, 3, 128], BF16, tag="wdd")
            nc.sync.dma_start(out=wdd[:], in_=wdd_d[:])
            idxt = cpool.tile([128, NCALL * 64], I16)
            nc.sync.dma_start(out=idxt[:], in_=idx_d[:])

            # all gathered sender features stay resident in SBUF
            sres = cpool.tile([128, NCH, C], F32, tag="sres")

            with tc.tile_pool(name="ohp", bufs=3) as ohp, \
                 tc.tile_pool(name="ap", bufs=4) as apool, \
                 tc.tile_pool(name="tp", bufs=6) as tpool, \
                 tc.tile_pool(name="wr", bufs=2) as wrp, \
                 tc.tile_pool(name="psm", bufs=1, space="PSUM") as psm, \
                 tc.tile_pool(name="psx", bufs=2, space="PSUM") as psx, \
                 tc.tile_pool(name="psa", bufs=2, space="PSUM") as psa, \
                 tc.tile_pool(name="pso", bufs=1, space="PSUM") as pso:

                # ---- phase 1: h = nf @ wup (nfT streamed per batch) ----
                with tc.tile_pool(name="hsb", bufs=3) as hsb:
                    NFULL = N_NODES // 128                    # 156 full tiles
                    for b in range(0, NFULL, 8):
                        nt = min(8, NFULL - b)
                        nft = hsb.tile([C, 8 * 128], BF16, tag="nft")
                        nc.sync.dma_start(
                            out=nft[:, :nt * 128],
                            in_=nfT_d[:, b * 128:(b + nt) * 128])
                        hb = hsb.tile([128, 8, C], F32, tag="hsb")
                        for t0 in range(0, nt, 4):
                            tn = min(4, nt - t0)
                            ps = psm.tile([128, 4, C], F32, tag="mlp")
                            for t in range(t0, t0 + tn):
                                nc.tensor.matmul(
                                    ps[:, t - t0, :],
                                    nft[:, t * 128:(t + 1) * 128],
                                    wup[:], start=(t == t0), stop=True,
                                    skip_group_check=True)
                            nc.vector.tensor_copy(hb[:, t0:t0 + tn, :],
                                                  ps[:, :tn, :])
                        nc.scalar.dma_start(
                            out=h_d[b * 128:(b + nt) * 128].rearrange(
                                "(t p) c -> p t c", p=128),
                            in_=hb[:, :nt, :])
                    rem = N_NODES - NFULL * 128               # 32 tail rows
                    if rem:
                        nft = hsb.tile([C, 8 * 128], BF16, tag="nft")
                        nc.sync.dma_start(out=nft[:, :rem],
                                          in_=nfT_d[:, NFULL * 128:])
                        ps = psm.tile([128, 4, C], F32, tag="mlp")
                        nc.tensor.matmul(ps[:rem, 0, :], nft[:, :rem],
                                         wup[:], start=True, stop=True)
                        hb = hsb.tile([128, 8, C], F32, tag="hsb")
                        nc.vector.tensor_copy(hb[:rem, 0, :], ps[:rem, 0, :])
                        nc.scalar.dma_start(out=h_d[NFULL * 128:],
                                            in_=hb[:rem, 0, :])

                # gathers are issued just-in-time in the pair loop so their
                # many small SDMA packets don't starve the ohY stream
                issued = [0]

                def issue_gathers(target):
                    while issued[0] < min(target, NCALL):
                        cidx = issued[0]
                        nc.gpsimd.dma_gather(
                            sres[:, cidx * 8:(cidx + 1) * 8, :], h_d[:],
                            idxt[:, cidx * 64:(cidx + 1) * 64],
                            1024, 1024, C, queue_num=cidx % 2)
                        issued[0] += 1

                # ---- main loop over sub-block pairs ----
                # acc layout (double-width, garbage quadrants unused):
                #   acc1 [128, 256]: l0 [0:64] (A=[0:64,0:32], B=[64:128,32:64])
                #                    l1 [64:256] (A=[0:64,64:160], B=[64:,160:])
                #   acc2 [128, 320]: l2 (A=[0:64,0:160], B=[64:128,160:320])
                LW = ((0, 32), (32, 128), (128, OHW))
                next_pg = 0
                a3t = {}
                for p in range(NPAIR):
                    issue_gathers((2 * (p + 3) * CH) // 8 + 1)
                    acc1 = psa.tile([128, 256], F32, tag="acc1")
                    acc2 = psa.tile([128, 320], F32, tag="acc2")
                    ohtp = ohp.tile([128, CH, 2 * OHW], BF16, tag="ohY")
                    nc.sync.dma_start(out=ohtp[:], in_=ohY_d[p])
                    for j in range(CH):
                        # t stationary for both halves: [128, (l, half, c)]
                        tt = tpool.tile([128, 3, 2, C], BF16, tag="t_all")
                        mix = psx.tile([128, 2, 3 * C], F32, tag="mix")
                        for half in range(2):
                            ch = (2 * p + half) * CH + j
                            G, sub = divmod(ch, 4)
                            pg, parity = divmod(G, 2)
                            while next_pg <= pg and next_pg < NPG:
                                a3t[next_pg] = _emit_mlp_pair(
                                    nc, apool, psm, next_pg, radT_d,
                                    w1d, w2d, w3d)
                                next_pg += 1
                            p0 = 64 * parity
                            nc.tensor.matmul(
                                mix[:, half, :],
                                a3t[pg][p0:p0 + 64, sub * 128:(sub + 1) * 128],
                                w4d[p0:p0 + 64], start=(half == 0), stop=True,
                                tile_position=(p0, 0),
                                skip_group_check=True)
                            mixv = mix[:, half, :].rearrange(
                                "p (i c) -> p i c", i=3)
                            sv = sres[:, ch, :].unsqueeze(1) \
                                .broadcast_to((128, 3, C))
                            nc.vector.tensor_tensor(tt[:, :, half, :], mixv,
                                                    sv, OP.mult)

                        # 3 scatter matmuls; moving = [ohY_A_l | ohY_B_l]
                        sp = j == CH - 1
                        nc.tensor.matmul(
                            acc1[:, 0:64], tt[:, 0, :, :],
                            ohtp[:, j, 0:64],
                            start=(j == 0), stop=sp, skip_group_check=True)
                        nc.tensor.matmul(
                            acc1[:, 64:256], tt[:, 1, :, :],
                            ohtp[:, j, 64:256],
                            start=False, stop=sp, skip_group_check=True)
                        nc.tensor.matmul(
                            acc2[:], tt[:, 2, :, :],
                            ohtp[:, j, 256:576],
                            start=(j == 0), stop=sp, skip_group_check=True)

                    # ---- pair wrap-up: linear_down + output ----
                    # collect valid quadrants into aggs [128, (l0,l1,l2)]
                    aggs = wrp.tile([128, OHW], BF16, tag="aggs")
                    srcs = [(acc1, 0, 0, 32), (acc1, 64, 32, 128),
                            (acc2, 0, 128, OHW)]
                    for l, (accs, a0, c0, c1) in enumerate(srcs):
                        w = c1 - c0
                        nc.scalar.copy(aggs[0:64, c0:c1],
                                       accs[0:64, a0:a0 + w])
                        nc.vector.tensor_copy(aggs[64:128, c0:c1],
                                              accs[64:128, a0 + w:a0 + 2 * w])
                    o = pso.tile([128, OHW], F32, tag="o")
                    for l, (c0, c1) in enumerate(LW):
                        nc.tensor.matmul(
                            o[:, c0:c1], wdd[:, l, :], aggs[:, c0:c1],
                            start=(l == 0), stop=True, skip_group_check=True)
                    osb = wrp.tile([128, OHW], F32, tag="osb")
                    nc.scalar.copy(osb[:], o[:])
                    nc.sync.dma_start(out=out_d[p], in_=osb[:])

    nc.compile()
    return nc


_CACHE = {}


def _get_program(CH):
    if CH not in _CACHE:
        _CACHE[CH] = _build(CH)
    return _CACHE[CH]


def _make_in_maps(prep, sw, node_feats):
    nfT = np.ascontiguousarray(np.asarray(node_feats, np.float32).T).astype(BF)
    maps = []
    for k in range(NCORES):
        maps.append({
            "nfT": nfT, "wup": sw["wup"], "w1d": sw["w1d"], "w2d": sw["w2d"],
            "w3d": sw["w3d"], "w4d": sw["w4d"], "wdd": sw["wdd"],
            "idx": prep["idx"][k], "ohY": prep["ohY"][k],
            "radT": prep["radT"][k],
        })
    return maps


def _assemble(results):
    out = np.empty((N_NODES, 9 * C), np.float32)
    for k in range(NCORES):
        oc = results[k]["outp"]                     # [NPAIR, 128, 288]
        # -> [NSB=80, 64, 288]
        ocs = oc.reshape(NPAIR, 2, 64, OHW).reshape(NSB, 64, OHW)
        nsb_real = (NS + SBN - 1) // SBN            # 79
        for s in range(nsb_real):
            nn = min(SBN, NS - s * SBN)
            M = ocs[s]                              # [64 d, 288]
            r0 = k * NS + s * SBN
            out[r0:r0 + nn, 0:C] = M[:, 0:nn].T
            m1 = M[:, 32:128].reshape(64, 3, SBN)   # [d, m, n]
            out[r0:r0 + nn, C:4 * C] = m1[:, :, :nn].transpose(2, 0, 1) \
                .reshape(nn, 3 * C)
            m2 = M[:, 128:OHW].reshape(64, 5, SBN)
            out[r0:r0 + nn, 4 * C:] = m2[:, :, :nn].transpose(2, 0, 1) \
                .reshape(nn, 5 * C)
    return out


def kernel(vectors, node_feats, radial_embedding, senders, receivers,
           w_up, mlp_w1, mlp_w2, mlp_w3, mlp_w4,
           w_down0, w_down1, w_down2):
    prep = _host_prep(vectors, node_feats, radial_embedding, senders, receivers)
    sw = _scaled_weights(w_up, mlp_w1, mlp_w2, mlp_w3, mlp_w4,
                         w_down0, w_down1, w_down2)
    nc = _get_program(prep["CH"])
    in_maps = _make_in_maps(prep, sw, node_feats)
    res = run_bass_kernel_spmd(nc, in_maps, list(range(NCORES)))
    return _assemble(res.results)


# revision 30
# speedup vs baseline: 1.5685x; 1.0050x over previous
"""Trainium2 Bass kernel for the GNN interaction layer (e3nn-style message passing).

Strategy: partition edges across 8 cores by receiver (2500 nodes/core), grouped
by 32-node receiver sub-blocks.  The spherical harmonics are folded into
host-precomputed bf16 "Y-scaled one-hot" matrices ohY[e, (l,m,n32)], streamed
from DRAM.  The scatter runs in swapped orientation on the PE: stationary =
per-edge gated features t_l (from the radial-MLP mix), moving = ohY, so the
per-node aggregate lands PSUM-transposed [c, (m,n)] and linear_down is a plain
per-irrep matmul with no transposes.  Two sub-blocks are processed concurrently
via tile_position column tiling.  Sender features are dma_gather'ed from a
device-computed h table; all gathers are issued up front so the Q7 descriptor
generation overlaps the main loop.  All matmuls are bf16.
"""
import math
import numpy as np
import ml_dtypes

from concourse import bacc, mybir, tile
from concourse.bass_utils import run_bass_kernel_spmd

F32 = mybir.dt.float32
BF16 = mybir.dt.bfloat16
I16 = mybir.dt.int16
AF = mybir.ActivationFunctionType
OP = mybir.AluOpType
BF = ml_dtypes.bfloat16

C = 64
R = 8
EPS = 0.5
N_NODES = 20000
N_EDGES = 320000
NCORES = 8
NS = N_NODES // NCORES          # nodes per core (2500)
SBN = 32                        # nodes per sub-block
NSB = 80                        # sub-blocks per core (79 real + 1 pad)
NPAIR = NSB // 2                # sub-block pairs (40)
# ohY column layout per chunk: l0 [0:32], l1 [32:128], l2 [128:288]
OHW = 288


def _spherical(v):
    u = v / np.linalg.norm(v, axis=-1, keepdims=True)
    x, y, z = u[:, 0], u[:, 1], u[:, 2]
    s15 = math.sqrt(15.0)
    y1 = math.sqrt(3.0) * u                                   # [E, 3]
    y2 = np.stack([
        s15 * x * y,
        s15 * y * z,
        0.5 * math.sqrt(5.0) * (3.0 * z * z - 1.0),
        s15 * x * z,
        0.5 * s15 * (x * x - y * y),
    ], axis=-1)                                               # [E, 5]
    return y1.astype(np.float32), y2.astype(np.float32)


def _host_prep(vectors, node_feats, radial, senders, receivers):
    senders = np.asarray(senders)
    receivers = np.asarray(receivers)
    vectors = np.asarray(vectors, np.float32)
    radial = np.asarray(radial, np.float32)

    core = receivers // NS
    rr = receivers % NS
    sb = rr // SBN                                # 0..78
    n32 = rr % SBN
    gkey = core * NSB + sb
    order = np.argsort(gkey, kind="stable")
    counts = np.bincount(gkey, minlength=NCORES * NSB)
    CH = max(2, int(math.ceil(counts.max() / 128.0)))
    SBW = CH * 128                                # slots per sub-block
    TOT = NSB * SBW                               # padded slots per core
    NCH = TOT // 128
    NG = TOT // 512
    NPG = NG // 2
    NCALL = TOT // 1024
    assert TOT % 1024 == 0

    # slot index for each edge (in sorted order)
    starts = np.concatenate([[0], np.cumsum(counts)])
    rank = np.arange(len(order)) - starts[gkey[order]]
    g_ord = gkey[order]
    slot = (g_ord % NSB) * SBW + rank             # slot within its core
    core_ord = g_ord // NSB

    y1, y2 = _spherical(vectors)
    y1o, y2o = y1[order], y2[order]
    n32o = n32[order]
    sndo = senders[order].astype(np.int16)
    rado = radial[order]

    snd = np.zeros((NCORES, TOT), np.int16)
    rad = np.zeros((NCORES, TOT, R), np.float32)
    snd[core_ord, slot] = sndo
    rad[core_ord, slot] = rado

    # ohY: [NCORES, TOT, 288] fp32 -> consumption-ordered bf16
    # one row per pair: [128, CH*2*OHW] (j-major, halves side by side)
    ohY_d = np.zeros((NCORES, NPAIR, 128, CH * 2 * OHW), BF)
    for k in range(NCORES):
        m = core_ord == k
        sl = slot[m]
        oh = np.zeros((TOT, OHW), np.float32)
        rows = sl
        oh[rows, n32o[m]] = 1.0
        for mm in range(3):
            oh[rows, 32 + 32 * mm + n32o[m]] = y1o[m, mm]
        for mm in range(5):
            oh[rows, 128 + 32 * mm + n32o[m]] = y2o[m, mm]
        # pair packing: per step j the 576 cols are l-grouped [A_l | B_l]:
        # l0: A32|B32, l1: A96|B96, l2: A160|B160
        ohp = oh.reshape(NPAIR, 2, CH, 128, OHW)
        dst = np.empty((NPAIR, 128, CH, 2 * OHW), np.float32)
        for l, (c0, c1) in enumerate(((0, 32), (32, 128), (128, OHW))):
            w = c1 - c0
            o0 = 2 * c0
            dst[:, :, :, o0:o0 + w] = ohp[:, 0, :, :, c0:c1].transpose(0, 2, 1, 3)
            dst[:, :, :, o0 + w:o0 + 2 * w] = \
                ohp[:, 1, :, :, c0:c1].transpose(0, 2, 1, 3)
        ohY_d[k] = dst.reshape(NPAIR, 128, CH * 2 * OHW).astype(BF)

    # gather idx: wrapped [16, 64] per 1024-slot call, tiled to 128 partitions
    idx = np.zeros((NCORES, 128, NCALL * 64), np.int16)
    for cidx in range(NCALL):
        blk = snd[:, cidx * 1024:(cidx + 1) * 1024]          # [NCORES, 1024]
        wrapped = blk.reshape(NCORES, 64, 16).transpose(0, 2, 1)
        idx[:, :, cidx * 64:(cidx + 1) * 64] = np.tile(wrapped, (1, 8, 1))

    # radial, transposed, packed per MLP pair: rows 0:8 even group, 8:16 odd
    radT = np.zeros((NCORES, 16, NPG * 512), BF)
    radt = rad.transpose(0, 2, 1)                            # [NCORES, R, TOT]
    for pg in range(NPG):
        radT[:, 0:8, pg * 512:(pg + 1) * 512] = \
            radt[:, :, (2 * pg) * 512:(2 * pg + 1) * 512]
        radT[:, 8:16, pg * 512:(pg + 1) * 512] = \
            radt[:, :, (2 * pg + 1) * 512:(2 * pg + 2) * 512]

    return dict(CH=CH, TOT=TOT, NCH=NCH, NG=NG, NPG=NPG, NCALL=NCALL,
                idx=idx, ohY=ohY_d, radT=radT)


def _scaled_weights(w_up, w1, w2, w3, w4, wd0, wd1, wd2):
    inv_sqrt_c = 1.0 / math.sqrt(C)
    w1s = (np.asarray(w1) / math.sqrt(R)).astype(np.float32)
    w2s = (np.asarray(w2) / math.sqrt(64.0)).astype(np.float32)
    w3s = (np.asarray(w3) / math.sqrt(64.0)).astype(np.float32)
    w4s = (np.asarray(w4) * (1.0 / math.sqrt(64.0)) * (1.0 / C)).astype(np.float32)
    w1d = np.zeros((128, 64), np.float32)
    w1d[0:R] = w1s
    w1d[64:64 + R] = w1s
    w2d = np.concatenate([w2s, w2s], axis=0)
    w3d = np.concatenate([w3s, w3s], axis=0)
    w4d = np.concatenate([w4s, w4s], axis=0)
    # block-diagonal per irrep: one matmul handles both halves
    wdd = np.zeros((128, 3, 128), np.float32)
    for i, wd in enumerate((wd0, wd1, wd2)):
        s = np.asarray(wd) * EPS * inv_sqrt_c
        wdd[0:64, i, 0:64] = s
        wdd[64:128, i, 64:128] = s
    return dict(
        wup=(np.asarray(w_up) * inv_sqrt_c).astype(BF),
        w1d=w1d.astype(BF), w2d=w2d.astype(BF), w3d=w3d.astype(BF),
        w4d=w4d.astype(BF), wdd=wdd.astype(BF),
    )


def _emit_mlp_pair(nc, apool, psm, pg, radT_d, w1d, w2d, w3d):
    """MLP layers 1-3 for groups 2*pg (partitions 0-63) and 2*pg+1 (64-127)."""
    rt = apool.tile([128, 512], BF16, tag="radT")
    nc.sync.dma_start(out=rt[0:R], in_=radT_d[0:8, pg * 512:(pg + 1) * 512])
    nc.scalar.dma_start(out=rt[64:64 + R],
                        in_=radT_d[8:16, pg * 512:(pg + 1) * 512])

    ps1 = psm.tile([128, 512], F32, tag="mlp")
    nc.tensor.matmul(ps1[0:64], w1d[0:R], rt[0:R], start=True, stop=True,
                     tile_position=(0, 0))
    nc.tensor.matmul(ps1[64:128], w1d[64:64 + R], rt[64:64 + R], start=True,
                     stop=True, tile_position=(64, 64))
    a1 = apool.tile([128, 512], BF16, tag="a1")
    nc.scalar.activation(a1[:], ps1[:], AF.Silu)

    ps2 = psm.tile([128, 512], F32, tag="mlp")
    nc.tensor.matmul(ps2[0:64], w2d[0:64], a1[0:64], start=True, stop=True,
                     tile_position=(0, 0))
    nc.tensor.matmul(ps2[64:128], w2d[64:128], a1[64:128], start=True,
                     stop=True, tile_position=(64, 64))
    a2 = apool.tile([128, 512], BF16, tag="a2")
    nc.scalar.activation(a2[:], ps2[:], AF.Silu)

    ps3 = psm.tile([128, 512], F32, tag="mlp")
    nc.tensor.matmul(ps3[0:64], w3d[0:64], a2[0:64], start=True, stop=True,
                     tile_position=(0, 0))
    nc.tensor.matmul(ps3[64:128], w3d[64:128], a2[64:128], start=True,
                     stop=True, tile_position=(64, 64))
    a3 = apool.tile([128, 512], BF16, tag="a3")
    nc.scalar.activation(a3[:], ps3[:], AF.Silu)
    return a3


def _build(CH):
    SBW = CH * 128
    TOT = NSB * SBW
    NCH = TOT // 128
    NG = TOT // 512
    NPG = NG // 2
    NCALL = TOT // 1024

    nc = bacc.Bacc(None, target_bir_lowering=False, debug=False,
                   dynamic_dma_scratch_size=16384, num_swdge_queues=2)

    nfT_d = nc.dram_tensor("nfT", [C, N_NODES], BF16, kind="ExternalInput")
    wup_d = nc.dram_tensor("wup", [C, C], BF16, kind="ExternalInput")
    w1_d = nc.dram_tensor("w1d", [128, 64], BF16, kind="ExternalInput")
    w2_d = nc.dram_tensor("w2d", [128, 64], BF16, kind="ExternalInput")
    w3_d = nc.dram_tensor("w3d", [128, 64], BF16, kind="ExternalInput")
    w4_d = nc.dram_tensor("w4d", [128, 3 * C], BF16, kind="ExternalInput")
    wdd_d = nc.dram_tensor("wdd", [128, 3, 128], BF16, kind="ExternalInput")
    idx_d = nc.dram_tensor("idx", [128, NCALL * 64], I16, kind="ExternalInput")
    ohY_d = nc.dram_tensor("ohY", [NPAIR, 128, CH * 2 * OHW], BF16,
                           kind="ExternalInput")
    radT_d = nc.dram_tensor("radT", [16, NPG * 512], BF16, kind="ExternalInput")

    h_d = nc.dram_tensor("h", [N_NODES, C], F32)
    out_d = nc.dram_tensor("outp", [NPAIR, 128, OHW], F32, kind="ExternalOutput")

    with tile.TileContext(nc) as tc:
        with tc.tile_pool(name="const", bufs=1) as cpool:
            wup = cpool.tile([C, C], BF16)
            nc.sync.dma_start(out=wup[:], in_=wup_d[:])
            w1d = cpool.tile([128, 64], BF16, tag="w1d")
            nc.sync.dma_start(out=w1d[:], in_=w1_d[:])
            w2d = cpool.tile([128, 64], BF16, tag="w2d")
            nc.sync.dma_start(out=w2d[:], in_=w2_d[:])
            w3d = cpool.tile([128, 64], BF16, tag="w3d")
            nc.sync.dma_start(out=w3d[:], in_=w3_d[:])
            w4d = cpool.tile([128, 3 * C], BF16, tag="w4d")
            nc.sync.dma_start(out=w4d[:], in_=w4_d[:])
            wdd = cpool.tile([128, 3, 128], BF16, tag="wdd")
            nc.sync.dma_start(out=wdd[:], in_=wdd_d[:])
            idxt = cpool.tile([128, NCALL * 64], I16)
            nc.sync.dma_start(out=idxt[:], in_=idx_d[:])

            # all gathered sender features stay resident in SBUF
            sres = cpool.tile([128, NCH, C], F32, tag="sres")

            with tc.tile_pool(name="ohp", bufs=3) as ohp, \
                 tc.tile_pool(name="ap", bufs=4) as apool, \
                 tc.tile_pool(name="tp", bufs=6) as tpool, \
                 tc.tile_pool(name="wr", bufs=2) as wrp, \
                 tc.tile_pool(name="psm", bufs=1, space="PSUM") as psm, \
                 tc.tile_pool(name="psx", bufs=2, space="PSUM") as psx, \
                 tc.tile_pool(name="psa", bufs=2, space="PSUM") as psa, \
                 tc.tile_pool(name="pso", bufs=1, space="PSUM") as pso:

                # ---- phase 1: h = nf @ wup (nfT streamed per batch) ----
                with tc.tile_pool(name="hsb", bufs=3) as hsb:
                    NFULL = N_NODES // 128                    # 156 full tiles
                    for b in range(0, NFULL, 8):
                        nt = min(8, NFULL - b)
                        nft = hsb.tile([C, 8 * 128], BF16, tag="nft")
                        nc.sync.dma_start(
                            out=nft[:, :nt * 128],
                            in_=nfT_d[:, b * 128:(b + nt) * 128])
                        hb = hsb.tile([128, 8, C], F32, tag="hsb")
                        for t0 in range(0, nt, 4):
                            tn = min(4, nt - t0)
                            ps = psm.tile([128, 4, C], F32, tag="mlp")
                            for t in range(t0, t0 + tn):
                                nc.tensor.matmul(
                                    ps[:, t - t0, :],
                                    nft[:, t * 128:(t + 1) * 128],
                                    wup[:], start=(t == t0), stop=True,
                                    skip_group_check=True)
                            nc.vector.tensor_copy(hb[:, t0:t0 + tn, :],
                                                  ps[:, :tn, :])
                        nc.scalar.dma_start(
                            out=h_d[b * 128:(b + nt) * 128].rearrange(
                                "(t p) c -> p t c", p=128),
                            in_=hb[:, :nt, :])
                    rem = N_NODES - NFULL * 128               # 32 tail rows
                    if rem:
                        nft = hsb.tile([C, 8 * 128], BF16, tag="nft")
                        nc.sync.dma_start(out=nft[:, :rem],
                                          in_=nfT_d[:, NFULL * 128:])
                        ps = psm.tile([128, 4, C], F32, tag="mlp")
                        nc.tensor.matmul(ps[:rem, 0, :], nft[:, :rem],
                                         wup[:], start=True, stop=True)
                        hb = hsb.tile([128, 8, C], F32, tag="hsb")
                        nc.vector.tensor_copy(hb[:rem, 0, :], ps[:rem, 0, :])
                        nc.scalar.dma_start(out=h_d[NFULL * 128:],
                                            in_=hb[:rem, 0, :])

                # gathers are issued just-in-time in the pair loop so their
                # many small SDMA packets don't starve the ohY stream
                issued = [0]

                def issue_gathers(target):
                    while issued[0] < min(target, NCALL):
                        cidx = issued[0]
                        nc.gpsimd.dma_gather(
                            sres[:, cidx * 8:(cidx + 1) * 8, :], h_d[:],
                            idxt[:, cidx * 64:(cidx + 1) * 64],
                            1024, 1024, C, queue_num=cidx % 2)
                        issued[0] += 1

                # ---- main loop over sub-block pairs ----
                # acc layout (double-width, garbage quadrants unused):
                #   acc1 [128, 256]: l0 [0:64] (A=[0:64,0:32], B=[64:128,32:64])
                #                    l1 [64:256] (A=[0:64,64:160], B=[64:,160:])
                #   acc2 [128, 320]: l2 (A=[0:64,0:160], B=[64:128,160:320])
                LW = ((0, 32), (32, 128), (128, OHW))
                next_pg = 0
                a3t = {}
                state = {"next_pg": 0}

                def emit_mixtt(p, j):
                    """mix matmuls + t_all TTs for step (p, j).  Gathers for
                    this step's chunks (plus lead) must be issued BEFORE the
                    sres reads appear in program order."""
                    issue_gathers((2 * (p + 3) * CH) // 8 + 1)
                    tt = tpool.tile([128, 3, 2, C], BF16, tag="t_all")
                    mix = psx.tile([128, 2, 3 * C], F32, tag="mix")
                    for half in range(2):
                        ch = (2 * p + half) * CH + j
                        G, sub = divmod(ch, 4)
                        pg, parity = divmod(G, 2)
                        while state["next_pg"] <= pg and state["next_pg"] < NPG:
                            a3t[state["next_pg"]] = _emit_mlp_pair(
                                nc, apool, psm, state["next_pg"], radT_d,
                                w1d, w2d, w3d)
                            state["next_pg"] += 1
                        p0 = 64 * parity
                        nc.tensor.matmul(
                            mix[:, half, :],
                            a3t[pg][p0:p0 + 64, sub * 128:(sub + 1) * 128],
                            w4d[p0:p0 + 64], start=(half == 0), stop=True,
                            tile_position=(p0, 0),
                            skip_group_check=True)
                        mixv = mix[:, half, :].rearrange(
                            "p (i c) -> p i c", i=3)
                        sv = sres[:, ch, :].unsqueeze(1) \
                            .broadcast_to((128, 3, C))
                        nc.vector.tensor_tensor(tt[:, :, half, :], mixv,
                                                sv, OP.mult)
                    return tt

                steps = [(p, j) for p in range(NPAIR) for j in range(CH)]
                tts = {0: emit_mixtt(*steps[0])}
                for si, (p, j) in enumerate(steps):
                    if j == 0:
                        acc1 = psa.tile([128, 256], F32, tag="acc1")
                        acc2 = psa.tile([128, 320], F32, tag="acc2")
                        ohtp = ohp.tile([128, CH, 2 * OHW], BF16, tag="ohY")
                        nc.sync.dma_start(out=ohtp[:], in_=ohY_d[p])
                    # software pipeline: mix+TT for the NEXT step ahead of
                    # this step's scatter so the PE queue never starves
                    if si + 1 < len(steps):
                        tts[si + 1] = emit_mixtt(*steps[si + 1])
                    tt = tts.pop(si)

                    # 3 scatter matmuls; moving = [ohY_A_l | ohY_B_l]
                    sp = j == CH - 1
                    nc.tensor.matmul(
                        acc1[:, 0:64], tt[:, 0, :, :],
                        ohtp[:, j, 0:64],
                        start=(j == 0), stop=sp, skip_group_check=True)
                    nc.tensor.matmul(
                        acc1[:, 64:256], tt[:, 1, :, :],
                        ohtp[:, j, 64:256],
                        start=False, stop=sp, skip_group_check=True)
                    nc.tensor.matmul(
                        acc2[:], tt[:, 2, :, :],
                        ohtp[:, j, 256:576],
                        start=(j == 0), stop=sp, skip_group_check=True)
                    if j != CH - 1:
                        continue

                    # ---- pair wrap-up: linear_down + output ----
                    # collect valid quadrants into aggs [128, (l0,l1,l2)]
                    aggs = wrp.tile([128, OHW], BF16, tag="aggs")
                    srcs = [(acc1, 0, 0, 32), (acc1, 64, 32, 128),
                            (acc2, 0, 128, OHW)]
                    for l, (accs, a0, c0, c1) in enumerate(srcs):
                        w = c1 - c0
                        nc.scalar.copy(aggs[0:64, c0:c1],
                                       accs[0:64, a0:a0 + w])
                        nc.vector.tensor_copy(aggs[64:128, c0:c1],
                                              accs[64:128, a0 + w:a0 + 2 * w])
                    o = pso.tile([128, OHW], F32, tag="o")
                    for l, (c0, c1) in enumerate(LW):
                        nc.tensor.matmul(
                            o[:, c0:c1], wdd[:, l, :], aggs[:, c0:c1],
                            start=(l == 0), stop=True, skip_group_check=True)
                    osb = wrp.tile([128, OHW], F32, tag="osb")
                    nc.scalar.copy(osb[:], o[:])
                    nc.sync.dma_start(out=out_d[p], in_=osb[:])

    nc.compile()
    return nc


_CACHE = {}


def _get_program(CH):
    if CH not in _CACHE:
        _CACHE[CH] = _build(CH)
    return _CACHE[CH]


def _make_in_maps(prep, sw, node_feats):
    nfT = np.ascontiguousarray(np.asarray(node_feats, np.float32).T).astype(BF)
    maps = []
    for k in range(NCORES):
        maps.append({
            "nfT": nfT, "wup": sw["wup"], "w1d": sw["w1d"], "w2d": sw["w2d"],
            "w3d": sw["w3d"], "w4d": sw["w4d"], "wdd": sw["wdd"],
            "idx": prep["idx"][k], "ohY": prep["ohY"][k],
            "radT": prep["radT"][k],
        })
    return maps


def _assemble(results):
    out = np.empty((N_NODES, 9 * C), np.float32)
    for k in range(NCORES):
        oc = results[k]["outp"]                     # [NPAIR, 128, 288]
        # -> [NSB=80, 64, 288]
        ocs = oc.reshape(NPAIR, 2, 64, OHW).reshape(NSB, 64, OHW)
        nsb_real = (NS + SBN - 1) // SBN            # 79
        for s in range(nsb_real):
            nn = min(SBN, NS - s * SBN)
            M = ocs[s]                              # [64 d, 288]
            r0 = k * NS + s * SBN
            out[r0:r0 + nn, 0:C] = M[:, 0:nn].T
            m1 = M[:, 32:128].reshape(64, 3, SBN)   # [d, m, n]
            out[r0:r0 + nn, C:4 * C] = m1[:, :, :nn].transpose(2, 0, 1) \
                .reshape(nn, 3 * C)
            m2 = M[:, 128:OHW].reshape(64, 5, SBN)
            out[r0:r0 + nn, 4 * C:] = m2[:, :, :nn].transpose(2, 0, 1) \
                .reshape(nn, 5 * C)
    return out


def kernel(vectors, node_feats, radial_embedding, senders, receivers,
           w_up, mlp_w1, mlp_w2, mlp_w3, mlp_w4,
           w_down0, w_down1, w_down2):
    prep = _host_prep(vectors, node_feats, radial_embedding, senders, receivers)
    sw = _scaled_weights(w_up, mlp_w1, mlp_w2, mlp_w3, mlp_w4,
                         w_down0, w_down1, w_down2)
    nc = _get_program(prep["CH"])
    in_maps = _make_in_maps(prep, sw, node_feats)
    res = run_bass_kernel_spmd(nc, in_maps, list(range(NCORES)))
    return _assemble(res.results)


# revision 31
# speedup vs baseline: 1.8257x; 1.1640x over previous
"""Trainium2 Bass kernel for the GNN interaction layer (e3nn-style message passing).

Strategy: partition edges across 8 cores by receiver (2500 nodes/core), grouped
by 32-node receiver sub-blocks.  The spherical harmonics are folded into
host-precomputed bf16 "Y-scaled one-hot" matrices ohY[e, (l,m,n32)], streamed
from DRAM.  The scatter runs in swapped orientation on the PE: stationary =
per-edge gated features t_l (from the radial-MLP mix), moving = ohY, so the
per-node aggregate lands PSUM-transposed [c, (m,n)] and linear_down is a plain
per-irrep matmul with no transposes.  Two sub-blocks are processed concurrently
via tile_position column tiling.  Sender features are dma_gather'ed from a
device-computed h table; all gathers are issued up front so the Q7 descriptor
generation overlaps the main loop.  All matmuls are bf16.
"""
import math
import numpy as np
import ml_dtypes

from concourse import bacc, mybir, tile
from concourse.bass_utils import run_bass_kernel_spmd

F32 = mybir.dt.float32
BF16 = mybir.dt.bfloat16
I16 = mybir.dt.int16
AF = mybir.ActivationFunctionType
OP = mybir.AluOpType
BF = ml_dtypes.bfloat16

C = 64
R = 8
EPS = 0.5
N_NODES = 20000
N_EDGES = 320000
NCORES = 8
NS = N_NODES // NCORES          # nodes per core (2500)
SBN = 32                        # nodes per sub-block
NSB = 80                        # sub-blocks per core (79 real + 1 pad)
NPAIR = NSB // 2                # sub-block pairs (40)
# ohY column layout per chunk: l0 [0:32], l1 [32:128], l2 [128:288]
OHW = 288


def _spherical(v):
    u = v / np.linalg.norm(v, axis=-1, keepdims=True)
    x, y, z = u[:, 0], u[:, 1], u[:, 2]
    s15 = math.sqrt(15.0)
    y1 = math.sqrt(3.0) * u                                   # [E, 3]
    y2 = np.stack([
        s15 * x * y,
        s15 * y * z,
        0.5 * math.sqrt(5.0) * (3.0 * z * z - 1.0),
        s15 * x * z,
        0.5 * s15 * (x * x - y * y),
    ], axis=-1)                                               # [E, 5]
    return y1.astype(np.float32), y2.astype(np.float32)


def _host_prep(vectors, node_feats, radial, senders, receivers):
    import heapq
    senders = np.asarray(senders)
    receivers = np.asarray(receivers)
    vectors = np.asarray(vectors, np.float32)
    radial = np.asarray(radial, np.float32)

    # degree-balanced assignment of nodes to (core, sub-block) bins so the
    # max per-sub-block edge count (hence CH) is minimal
    NBIN = NCORES * NSB
    deg = np.bincount(receivers, minlength=N_NODES)
    node_bin = np.empty(N_NODES, np.int64)
    node_pos = np.empty(N_NODES, np.int64)
    heap = [(0, 0, b) for b in range(NBIN)]
    heapq.heapify(heap)
    for n in np.argsort(-deg, kind="stable"):
        while True:
            load, cnt, b = heapq.heappop(heap)
            if cnt < SBN:
                break
        node_bin[n] = b
        node_pos[n] = cnt
        heapq.heappush(heap, (load + int(deg[n]), cnt + 1, b))

    ebin = node_bin[receivers]
    n32 = node_pos[receivers]
    gkey = ebin
    order = np.argsort(gkey, kind="stable")
    counts = np.bincount(gkey, minlength=NCORES * NSB)
    CH = max(2, int(math.ceil(counts.max() / 128.0)))
    SBW = CH * 128                                # slots per sub-block
    TOT = NSB * SBW                               # padded slots per core
    NCH = TOT // 128
    NG = TOT // 512
    NPG = NG // 2
    NCALL = TOT // 1024
    assert TOT % 1024 == 0

    # slot index for each edge (in sorted order)
    starts = np.concatenate([[0], np.cumsum(counts)])
    rank = np.arange(len(order)) - starts[gkey[order]]
    g_ord = gkey[order]
    slot = (g_ord % NSB) * SBW + rank             # slot within its core
    core_ord = g_ord // NSB

    y1, y2 = _spherical(vectors)
    y1o, y2o = y1[order], y2[order]
    n32o = n32[order]
    sndo = senders[order].astype(np.int16)
    rado = radial[order]

    snd = np.zeros((NCORES, TOT), np.int16)
    rad = np.zeros((NCORES, TOT, R), np.float32)
    snd[core_ord, slot] = sndo
    rad[core_ord, slot] = rado

    # ohY: [NCORES, TOT, 288] fp32 -> consumption-ordered bf16
    # one row per pair: [128, CH*2*OHW] (j-major, halves side by side)
    ohY_d = np.zeros((NCORES, NPAIR, 128, CH * 2 * OHW), BF)
    for k in range(NCORES):
        m = core_ord == k
        sl = slot[m]
        oh = np.zeros((TOT, OHW), np.float32)
        rows = sl
        oh[rows, n32o[m]] = 1.0
        for mm in range(3):
            oh[rows, 32 + 32 * mm + n32o[m]] = y1o[m, mm]
        for mm in range(5):
            oh[rows, 128 + 32 * mm + n32o[m]] = y2o[m, mm]
        # pair packing: per step j the 576 cols are l-grouped [A_l | B_l]:
        # l0: A32|B32, l1: A96|B96, l2: A160|B160
        ohp = oh.reshape(NPAIR, 2, CH, 128, OHW)
        dst = np.empty((NPAIR, 128, CH, 2 * OHW), np.float32)
        for l, (c0, c1) in enumerate(((0, 32), (32, 128), (128, OHW))):
            w = c1 - c0
            o0 = 2 * c0
            dst[:, :, :, o0:o0 + w] = ohp[:, 0, :, :, c0:c1].transpose(0, 2, 1, 3)
            dst[:, :, :, o0 + w:o0 + 2 * w] = \
                ohp[:, 1, :, :, c0:c1].transpose(0, 2, 1, 3)
        ohY_d[k] = dst.reshape(NPAIR, 128, CH * 2 * OHW).astype(BF)

    # gather idx: wrapped [16, 64] per 1024-slot call, tiled to 128 partitions
    idx = np.zeros((NCORES, 128, NCALL * 64), np.int16)
    for cidx in range(NCALL):
        blk = snd[:, cidx * 1024:(cidx + 1) * 1024]          # [NCORES, 1024]
        wrapped = blk.reshape(NCORES, 64, 16).transpose(0, 2, 1)
        idx[:, :, cidx * 64:(cidx + 1) * 64] = np.tile(wrapped, (1, 8, 1))

    # radial, transposed, packed per MLP pair: rows 0:8 even group, 8:16 odd
    radT = np.zeros((NCORES, 16, NPG * 512), BF)
    radt = rad.transpose(0, 2, 1)                            # [NCORES, R, TOT]
    for pg in range(NPG):
        radT[:, 0:8, pg * 512:(pg + 1) * 512] = \
            radt[:, :, (2 * pg) * 512:(2 * pg + 1) * 512]
        radT[:, 8:16, pg * 512:(pg + 1) * 512] = \
            radt[:, :, (2 * pg + 1) * 512:(2 * pg + 2) * 512]

    return dict(CH=CH, TOT=TOT, NCH=NCH, NG=NG, NPG=NPG, NCALL=NCALL,
                idx=idx, ohY=ohY_d, radT=radT,
                node_bin=node_bin, node_pos=node_pos)


def _scaled_weights(w_up, w1, w2, w3, w4, wd0, wd1, wd2):
    inv_sqrt_c = 1.0 / math.sqrt(C)
    w1s = (np.asarray(w1) / math.sqrt(R)).astype(np.float32)
    w2s = (np.asarray(w2) / math.sqrt(64.0)).astype(np.float32)
    w3s = (np.asarray(w3) / math.sqrt(64.0)).astype(np.float32)
    w4s = (np.asarray(w4) * (1.0 / math.sqrt(64.0)) * (1.0 / C)).astype(np.float32)
    w1d = np.zeros((128, 64), np.float32)
    w1d[0:R] = w1s
    w1d[64:64 + R] = w1s
    w2d = np.concatenate([w2s, w2s], axis=0)
    w3d = np.concatenate([w3s, w3s], axis=0)
    w4d = np.concatenate([w4s, w4s], axis=0)
    # block-diagonal per irrep: one matmul handles both halves
    wdd = np.zeros((128, 3, 128), np.float32)
    for i, wd in enumerate((wd0, wd1, wd2)):
        s = np.asarray(wd) * EPS * inv_sqrt_c
        wdd[0:64, i, 0:64] = s
        wdd[64:128, i, 64:128] = s
    return dict(
        wup=(np.asarray(w_up) * inv_sqrt_c).astype(BF),
        w1d=w1d.astype(BF), w2d=w2d.astype(BF), w3d=w3d.astype(BF),
        w4d=w4d.astype(BF), wdd=wdd.astype(BF),
    )


def _emit_mlp_pair(nc, apool, psm, pg, radT_d, w1d, w2d, w3d):
    """MLP layers 1-3 for groups 2*pg (partitions 0-63) and 2*pg+1 (64-127)."""
    rt = apool.tile([128, 512], BF16, tag="radT")
    nc.sync.dma_start(out=rt[0:R], in_=radT_d[0:8, pg * 512:(pg + 1) * 512])
    nc.scalar.dma_start(out=rt[64:64 + R],
                        in_=radT_d[8:16, pg * 512:(pg + 1) * 512])

    ps1 = psm.tile([128, 512], F32, tag="mlp")
    nc.tensor.matmul(ps1[0:64], w1d[0:R], rt[0:R], start=True, stop=True,
                     tile_position=(0, 0))
    nc.tensor.matmul(ps1[64:128], w1d[64:64 + R], rt[64:64 + R], start=True,
                     stop=True, tile_position=(64, 64))
    a1 = apool.tile([128, 512], BF16, tag="a1")
    nc.scalar.activation(a1[:], ps1[:], AF.Silu)

    ps2 = psm.tile([128, 512], F32, tag="mlp")
    nc.tensor.matmul(ps2[0:64], w2d[0:64], a1[0:64], start=True, stop=True,
                     tile_position=(0, 0))
    nc.tensor.matmul(ps2[64:128], w2d[64:128], a1[64:128], start=True,
                     stop=True, tile_position=(64, 64))
    a2 = apool.tile([128, 512], BF16, tag="a2")
    nc.scalar.activation(a2[:], ps2[:], AF.Silu)

    ps3 = psm.tile([128, 512], F32, tag="mlp")
    nc.tensor.matmul(ps3[0:64], w3d[0:64], a2[0:64], start=True, stop=True,
                     tile_position=(0, 0))
    nc.tensor.matmul(ps3[64:128], w3d[64:128], a2[64:128], start=True,
                     stop=True, tile_position=(64, 64))
    a3 = apool.tile([128, 512], BF16, tag="a3")
    nc.scalar.activation(a3[:], ps3[:], AF.Silu)
    return a3


def _build(CH):
    SBW = CH * 128
    TOT = NSB * SBW
    NCH = TOT // 128
    NG = TOT // 512
    NPG = NG // 2
    NCALL = TOT // 1024

    nc = bacc.Bacc(None, target_bir_lowering=False, debug=False,
                   dynamic_dma_scratch_size=16384, num_swdge_queues=2)

    nfT_d = nc.dram_tensor("nfT", [C, N_NODES], BF16, kind="ExternalInput")
    wup_d = nc.dram_tensor("wup", [C, C], BF16, kind="ExternalInput")
    w1_d = nc.dram_tensor("w1d", [128, 64], BF16, kind="ExternalInput")
    w2_d = nc.dram_tensor("w2d", [128, 64], BF16, kind="ExternalInput")
    w3_d = nc.dram_tensor("w3d", [128, 64], BF16, kind="ExternalInput")
    w4_d = nc.dram_tensor("w4d", [128, 3 * C], BF16, kind="ExternalInput")
    wdd_d = nc.dram_tensor("wdd", [128, 3, 128], BF16, kind="ExternalInput")
    idx_d = nc.dram_tensor("idx", [128, NCALL * 64], I16, kind="ExternalInput")
    ohY_d = nc.dram_tensor("ohY", [NPAIR, 128, CH * 2 * OHW], BF16,
                           kind="ExternalInput")
    radT_d = nc.dram_tensor("radT", [16, NPG * 512], BF16, kind="ExternalInput")

    h_d = nc.dram_tensor("h", [N_NODES, C], F32)
    out_d = nc.dram_tensor("outp", [NPAIR, 128, OHW], F32, kind="ExternalOutput")

    with tile.TileContext(nc) as tc:
        with tc.tile_pool(name="const", bufs=1) as cpool:
            wup = cpool.tile([C, C], BF16)
            nc.sync.dma_start(out=wup[:], in_=wup_d[:])
            w1d = cpool.tile([128, 64], BF16, tag="w1d")
            nc.sync.dma_start(out=w1d[:], in_=w1_d[:])
            w2d = cpool.tile([128, 64], BF16, tag="w2d")
            nc.sync.dma_start(out=w2d[:], in_=w2_d[:])
            w3d = cpool.tile([128, 64], BF16, tag="w3d")
            nc.sync.dma_start(out=w3d[:], in_=w3_d[:])
            w4d = cpool.tile([128, 3 * C], BF16, tag="w4d")
            nc.sync.dma_start(out=w4d[:], in_=w4_d[:])
            wdd = cpool.tile([128, 3, 128], BF16, tag="wdd")
            nc.sync.dma_start(out=wdd[:], in_=wdd_d[:])
            idxt = cpool.tile([128, NCALL * 64], I16)
            nc.sync.dma_start(out=idxt[:], in_=idx_d[:])

            # all gathered sender features stay resident in SBUF
            sres = cpool.tile([128, NCH, C], F32, tag="sres")

            with tc.tile_pool(name="ohp", bufs=3) as ohp, \
                 tc.tile_pool(name="ap", bufs=4) as apool, \
                 tc.tile_pool(name="tp", bufs=6) as tpool, \
                 tc.tile_pool(name="wr", bufs=2) as wrp, \
                 tc.tile_pool(name="psm", bufs=1, space="PSUM") as psm, \
                 tc.tile_pool(name="psx", bufs=2, space="PSUM") as psx, \
                 tc.tile_pool(name="psa", bufs=2, space="PSUM") as psa, \
                 tc.tile_pool(name="pso", bufs=1, space="PSUM") as pso:

                # ---- phase 1: h = nf @ wup (nfT streamed per batch) ----
                with tc.tile_pool(name="hsb", bufs=3) as hsb:
                    NFULL = N_NODES // 128                    # 156 full tiles
                    for b in range(0, NFULL, 8):
                        nt = min(8, NFULL - b)
                        nft = hsb.tile([C, 8 * 128], BF16, tag="nft")
                        nc.sync.dma_start(
                            out=nft[:, :nt * 128],
                            in_=nfT_d[:, b * 128:(b + nt) * 128])
                        hb = hsb.tile([128, 8, C], F32, tag="hsb")
                        for t0 in range(0, nt, 4):
                            tn = min(4, nt - t0)
                            ps = psm.tile([128, 4, C], F32, tag="mlp")
                            for t in range(t0, t0 + tn):
                                nc.tensor.matmul(
                                    ps[:, t - t0, :],
                                    nft[:, t * 128:(t + 1) * 128],
                                    wup[:], start=(t == t0), stop=True,
                                    skip_group_check=True)
                            nc.vector.tensor_copy(hb[:, t0:t0 + tn, :],
                                                  ps[:, :tn, :])
                        nc.scalar.dma_start(
                            out=h_d[b * 128:(b + nt) * 128].rearrange(
                                "(t p) c -> p t c", p=128),
                            in_=hb[:, :nt, :])
                    rem = N_NODES - NFULL * 128               # 32 tail rows
                    if rem:
                        nft = hsb.tile([C, 8 * 128], BF16, tag="nft")
                        nc.sync.dma_start(out=nft[:, :rem],
                                          in_=nfT_d[:, NFULL * 128:])
                        ps = psm.tile([128, 4, C], F32, tag="mlp")
                        nc.tensor.matmul(ps[:rem, 0, :], nft[:, :rem],
                                         wup[:], start=True, stop=True)
                        hb = hsb.tile([128, 8, C], F32, tag="hsb")
                        nc.vector.tensor_copy(hb[:rem, 0, :], ps[:rem, 0, :])
                        nc.scalar.dma_start(out=h_d[NFULL * 128:],
                                            in_=hb[:rem, 0, :])

                # gathers are issued just-in-time in the pair loop so their
                # many small SDMA packets don't starve the ohY stream
                issued = [0]

                def issue_gathers(target):
                    while issued[0] < min(target, NCALL):
                        cidx = issued[0]
                        nc.gpsimd.dma_gather(
                            sres[:, cidx * 8:(cidx + 1) * 8, :], h_d[:],
                            idxt[:, cidx * 64:(cidx + 1) * 64],
                            1024, 1024, C, queue_num=cidx % 2)
                        issued[0] += 1

                # ---- main loop over sub-block pairs ----
                # acc layout (double-width, garbage quadrants unused):
                #   acc1 [128, 256]: l0 [0:64] (A=[0:64,0:32], B=[64:128,32:64])
                #                    l1 [64:256] (A=[0:64,64:160], B=[64:,160:])
                #   acc2 [128, 320]: l2 (A=[0:64,0:160], B=[64:128,160:320])
                LW = ((0, 32), (32, 128), (128, OHW))
                next_pg = 0
                a3t = {}
                state = {"next_pg": 0}

                def emit_mixtt(p, j):
                    """mix matmuls + t_all TTs for step (p, j).  Gathers for
                    this step's chunks (plus lead) must be issued BEFORE the
                    sres reads appear in program order."""
                    issue_gathers((2 * (p + 3) * CH) // 8 + 1)
                    tt = tpool.tile([128, 3, 2, C], BF16, tag="t_all")
                    mix = psx.tile([128, 2, 3 * C], F32, tag="mix")
                    for half in range(2):
                        ch = (2 * p + half) * CH + j
                        G, sub = divmod(ch, 4)
                        pg, parity = divmod(G, 2)
                        while state["next_pg"] <= pg and state["next_pg"] < NPG:
                            a3t[state["next_pg"]] = _emit_mlp_pair(
                                nc, apool, psm, state["next_pg"], radT_d,
                                w1d, w2d, w3d)
                            state["next_pg"] += 1
                        p0 = 64 * parity
                        nc.tensor.matmul(
                            mix[:, half, :],
                            a3t[pg][p0:p0 + 64, sub * 128:(sub + 1) * 128],
                            w4d[p0:p0 + 64], start=(half == 0), stop=True,
                            tile_position=(p0, 0),
                            skip_group_check=True)
                        mixv = mix[:, half, :].rearrange(
                            "p (i c) -> p i c", i=3)
                        sv = sres[:, ch, :].unsqueeze(1) \
                            .broadcast_to((128, 3, C))
                        nc.vector.tensor_tensor(tt[:, :, half, :], mixv,
                                                sv, OP.mult)
                    return tt

                steps = [(p, j) for p in range(NPAIR) for j in range(CH)]
                tts = {0: emit_mixtt(*steps[0])}
                for si, (p, j) in enumerate(steps):
                    if j == 0:
                        acc1 = psa.tile([128, 256], F32, tag="acc1")
                        acc2 = psa.tile([128, 320], F32, tag="acc2")
                        ohtp = ohp.tile([128, CH, 2 * OHW], BF16, tag="ohY")
                        nc.sync.dma_start(out=ohtp[:], in_=ohY_d[p])
                    # software pipeline: mix+TT for the NEXT step ahead of
                    # this step's scatter so the PE queue never starves
                    if si + 1 < len(steps):
                        tts[si + 1] = emit_mixtt(*steps[si + 1])
                    tt = tts.pop(si)

                    # 3 scatter matmuls; moving = [ohY_A_l | ohY_B_l]
                    sp = j == CH - 1
                    nc.tensor.matmul(
                        acc1[:, 0:64], tt[:, 0, :, :],
                        ohtp[:, j, 0:64],
                        start=(j == 0), stop=sp, skip_group_check=True)
                    nc.tensor.matmul(
                        acc1[:, 64:256], tt[:, 1, :, :],
                        ohtp[:, j, 64:256],
                        start=False, stop=sp, skip_group_check=True)
                    nc.tensor.matmul(
                        acc2[:], tt[:, 2, :, :],
                        ohtp[:, j, 256:576],
                        start=(j == 0), stop=sp, skip_group_check=True)
                    if j != CH - 1:
                        continue

                    # ---- pair wrap-up: linear_down + output ----
                    # collect valid quadrants into aggs [128, (l0,l1,l2)]
                    aggs = wrp.tile([128, OHW], BF16, tag="aggs")
                    srcs = [(acc1, 0, 0, 32), (acc1, 64, 32, 128),
                            (acc2, 0, 128, OHW)]
                    for l, (accs, a0, c0, c1) in enumerate(srcs):
                        w = c1 - c0
                        nc.scalar.copy(aggs[0:64, c0:c1],
                                       accs[0:64, a0:a0 + w])
                        nc.vector.tensor_copy(aggs[64:128, c0:c1],
                                              accs[64:128, a0 + w:a0 + 2 * w])
                    o = pso.tile([128, OHW], F32, tag="o")
                    for l, (c0, c1) in enumerate(LW):
                        nc.tensor.matmul(
                            o[:, c0:c1], wdd[:, l, :], aggs[:, c0:c1],
                            start=(l == 0), stop=True, skip_group_check=True)
                    osb = wrp.tile([128, OHW], F32, tag="osb")
                    nc.scalar.copy(osb[:], o[:])
                    nc.sync.dma_start(out=out_d[p], in_=osb[:])

    nc.compile()
    return nc


_CACHE = {}


def _get_program(CH):
    if CH not in _CACHE:
        _CACHE[CH] = _build(CH)
    return _CACHE[CH]


def _make_in_maps(prep, sw, node_feats):
    nfT = np.ascontiguousarray(np.asarray(node_feats, np.float32).T).astype(BF)
    maps = []
    for k in range(NCORES):
        maps.append({
            "nfT": nfT, "wup": sw["wup"], "w1d": sw["w1d"], "w2d": sw["w2d"],
            "w3d": sw["w3d"], "w4d": sw["w4d"], "wdd": sw["wdd"],
            "idx": prep["idx"][k], "ohY": prep["ohY"][k],
            "radT": prep["radT"][k],
        })
    return maps


def _assemble(results, node_bin, node_pos):
    out = np.empty((N_NODES, 9 * C), np.float32)
    # A[k, s, d, w] for bin = k*NSB + s
    A = np.stack([r["outp"].reshape(NPAIR, 2, 64, OHW).reshape(NSB, 64, OHW)
                  for r in results])                # [8, NSB, 64, 288]
    A = A.reshape(NCORES * NSB, 64, OHW)            # [NBIN, 64, 288]
    bb = node_bin
    ii = node_pos
    out[:, 0:C] = A[bb, :, ii]                      # [N, 64]
    o1 = np.empty((N_NODES, C, 3), np.float32)
    for m in range(3):
        o1[:, :, m] = A[bb, :, 32 + 32 * m + ii]
    out[:, C:4 * C] = o1.reshape(N_NODES, 3 * C)
    o2 = np.empty((N_NODES, C, 5), np.float32)
    for m in range(5):
        o2[:, :, m] = A[bb, :, 128 + 32 * m + ii]
    out[:, 4 * C:] = o2.reshape(N_NODES, 5 * C)
    return out


def kernel(vectors, node_feats, radial_embedding, senders, receivers,
           w_up, mlp_w1, mlp_w2, mlp_w3, mlp_w4,
           w_down0, w_down1, w_down2):
    prep = _host_prep(vectors, node_feats, radial_embedding, senders, receivers)
    sw = _scaled_weights(w_up, mlp_w1, mlp_w2, mlp_w3, mlp_w4,
                         w_down0, w_down1, w_down2)
    nc = _get_program(prep["CH"])
    in_maps = _make_in_maps(prep, sw, node_feats)
    res = run_bass_kernel_spmd(nc, in_maps, list(range(NCORES)))
    return _assemble(res.results, prep["node_bin"], prep["node_pos"])


# revision 35
# speedup vs baseline: 1.8486x; 1.0126x over previous
"""Trainium2 Bass kernel for the GNN interaction layer (e3nn-style message passing).

Strategy: receiver nodes are packed degree-balanced into 8 cores x 80
sub-blocks of <=32 nodes (greedy LPT), minimizing the padded chunks-per-
sub-block CH (=4 for uniform random graphs, ~2% padding).  The spherical
harmonics are folded into host-precomputed bf16 "Y-scaled one-hot" matrices
ohY[e, (l, m, n32)], streamed from DRAM.  The scatter runs in swapped
orientation on the PE: stationary = per-edge gated features t_l (from the
radial-MLP mix), packed for two sub-blocks side by side in one 128-col
stationary; moving = [ohY_A | ohY_B], so one matmul per irrep covers both
sub-blocks (garbage quadrants accumulate in unused PSUM regions) and the
aggregate lands PSUM-transposed [c, (m,n)] so linear_down is a single
block-diagonal matmul per irrep with no transposes.  Sender features are
dma_gather'ed from a device-computed h table on two SWDGE queues, issued
just-in-time so the small gather packets don't starve the ohY DMA stream;
mix+t for step s+1 are emitted ahead of step s's scatter to keep the PE
queue fed.  All matmuls are bf16.
"""
import math
import numpy as np
import ml_dtypes

from concourse import bacc, mybir, tile
from concourse.bass_utils import run_bass_kernel_spmd

F32 = mybir.dt.float32
BF16 = mybir.dt.bfloat16
I16 = mybir.dt.int16
AF = mybir.ActivationFunctionType
OP = mybir.AluOpType
BF = ml_dtypes.bfloat16

C = 64
R = 8
EPS = 0.5
N_NODES = 20000
N_EDGES = 320000
NCORES = 8
NS = N_NODES // NCORES          # nodes per core (2500)
SBN = 32                        # nodes per sub-block
NSB = 80                        # sub-blocks per core (79 real + 1 pad)
NPAIR = NSB // 2                # sub-block pairs (40)
# ohY column layout per chunk: l0 [0:32], l1 [32:128], l2 [128:288]
OHW = 288


def _spherical(v):
    u = v / np.linalg.norm(v, axis=-1, keepdims=True)
    x, y, z = u[:, 0], u[:, 1], u[:, 2]
    s15 = math.sqrt(15.0)
    y1 = math.sqrt(3.0) * u                                   # [E, 3]
    y2 = np.stack([
        s15 * x * y,
        s15 * y * z,
        0.5 * math.sqrt(5.0) * (3.0 * z * z - 1.0),
        s15 * x * z,
        0.5 * s15 * (x * x - y * y),
    ], axis=-1)                                               # [E, 5]
    return y1.astype(np.float32), y2.astype(np.float32)


def _host_prep(vectors, node_feats, radial, senders, receivers):
    import heapq
    senders = np.asarray(senders)
    receivers = np.asarray(receivers)
    vectors = np.asarray(vectors, np.float32)
    radial = np.asarray(radial, np.float32)

    # degree-balanced assignment of nodes to (core, sub-block) bins so the
    # max per-sub-block edge count (hence CH) is minimal
    NBIN = NCORES * NSB
    deg = np.bincount(receivers, minlength=N_NODES)
    node_bin = np.empty(N_NODES, np.int64)
    node_pos = np.empty(N_NODES, np.int64)
    heap = [(0, 0, b) for b in range(NBIN)]
    heapq.heapify(heap)
    for n in np.argsort(-deg, kind="stable"):
        while True:
            load, cnt, b = heapq.heappop(heap)
            if cnt < SBN:
                break
        node_bin[n] = b
        node_pos[n] = cnt
        heapq.heappush(heap, (load + int(deg[n]), cnt + 1, b))

    ebin = node_bin[receivers]
    n32 = node_pos[receivers]
    gkey = ebin
    order = np.argsort(gkey, kind="stable")
    counts = np.bincount(gkey, minlength=NCORES * NSB)
    CH = max(2, int(math.ceil(counts.max() / 128.0)))
    SBW = CH * 128                                # slots per sub-block
    TOT = NSB * SBW                               # padded slots per core
    NCH = TOT // 128
    NG = TOT // 512
    NPG = NG // 2
    NCALL = TOT // 1024
    assert TOT % 1024 == 0

    # slot index for each edge (in sorted order)
    starts = np.concatenate([[0], np.cumsum(counts)])
    rank = np.arange(len(order)) - starts[gkey[order]]
    g_ord = gkey[order]
    slot = (g_ord % NSB) * SBW + rank             # slot within its core
    core_ord = g_ord // NSB

    y1, y2 = _spherical(vectors)
    y1o, y2o = y1[order], y2[order]
    n32o = n32[order]
    sndo = senders[order].astype(np.int16)
    rado = radial[order]

    snd = np.zeros((NCORES, TOT), np.int16)
    rad = np.zeros((NCORES, TOT, R), np.float32)
    snd[core_ord, slot] = sndo
    rad[core_ord, slot] = rado

    # ohY: [NCORES, TOT, 288] fp32 -> consumption-ordered bf16
    # one row per pair: [128, CH*2*OHW] (j-major, halves side by side)
    ohY_d = np.zeros((NCORES, NPAIR, 128, CH * 2 * OHW), BF)
    for k in range(NCORES):
        m = core_ord == k
        sl = slot[m]
        oh = np.zeros((TOT, OHW), np.float32)
        rows = sl
        oh[rows, n32o[m]] = 1.0
        for mm in range(3):
            oh[rows, 32 + 32 * mm + n32o[m]] = y1o[m, mm]
        for mm in range(5):
            oh[rows, 128 + 32 * mm + n32o[m]] = y2o[m, mm]
        # pair packing: per step j the 576 cols are l-grouped [A_l | B_l]:
        # l0: A32|B32, l1: A96|B96, l2: A160|B160
        ohp = oh.reshape(NPAIR, 2, CH, 128, OHW)
        dst = np.empty((NPAIR, 128, CH, 2 * OHW), np.float32)
        for l, (c0, c1) in enumerate(((0, 32), (32, 128), (128, OHW))):
            w = c1 - c0
            o0 = 2 * c0
            dst[:, :, :, o0:o0 + w] = ohp[:, 0, :, :, c0:c1].transpose(0, 2, 1, 3)
            dst[:, :, :, o0 + w:o0 + 2 * w] = \
                ohp[:, 1, :, :, c0:c1].transpose(0, 2, 1, 3)
        ohY_d[k] = dst.reshape(NPAIR, 128, CH * 2 * OHW).astype(BF)

    # gather idx: wrapped [16, 64] per 1024-slot call, tiled to 128 partitions
    idx = np.zeros((NCORES, 128, NCALL * 64), np.int16)
    for cidx in range(NCALL):
        blk = snd[:, cidx * 1024:(cidx + 1) * 1024]          # [NCORES, 1024]
        wrapped = blk.reshape(NCORES, 64, 16).transpose(0, 2, 1)
        idx[:, :, cidx * 64:(cidx + 1) * 64] = np.tile(wrapped, (1, 8, 1))

    # radial, transposed, packed per MLP pair: rows 0:8 even group, 8:16 odd
    radT = np.zeros((NCORES, 16, NPG * 512), BF)
    radt = rad.transpose(0, 2, 1)                            # [NCORES, R, TOT]
    for pg in range(NPG):
        radT[:, 0:8, pg * 512:(pg + 1) * 512] = \
            radt[:, :, (2 * pg) * 512:(2 * pg + 1) * 512]
        radT[:, 8:16, pg * 512:(pg + 1) * 512] = \
            radt[:, :, (2 * pg + 1) * 512:(2 * pg + 2) * 512]

    return dict(CH=CH, TOT=TOT, NCH=NCH, NG=NG, NPG=NPG, NCALL=NCALL,
                idx=idx, ohY=ohY_d, radT=radT,
                node_bin=node_bin, node_pos=node_pos)


def _scaled_weights(w_up, w1, w2, w3, w4, wd0, wd1, wd2):
    inv_sqrt_c = 1.0 / math.sqrt(C)
    w1s = (np.asarray(w1) / math.sqrt(R)).astype(np.float32)
    w2s = (np.asarray(w2) / math.sqrt(64.0)).astype(np.float32)
    w3s = (np.asarray(w3) / math.sqrt(64.0)).astype(np.float32)
    w4s = (np.asarray(w4) * (1.0 / math.sqrt(64.0)) * (1.0 / C)).astype(np.float32)
    w1d = np.zeros((128, 64), np.float32)
    w1d[0:R] = w1s
    w1d[64:64 + R] = w1s
    w2d = np.concatenate([w2s, w2s], axis=0)
    w3d = np.concatenate([w3s, w3s], axis=0)
    w4d = np.concatenate([w4s, w4s], axis=0)
    # block-diagonal per irrep: one matmul handles both halves
    wdd = np.zeros((128, 3, 128), np.float32)
    for i, wd in enumerate((wd0, wd1, wd2)):
        s = np.asarray(wd) * EPS * inv_sqrt_c
        wdd[0:64, i, 0:64] = s
        wdd[64:128, i, 64:128] = s
    return dict(
        wup=(np.asarray(w_up) * inv_sqrt_c).astype(BF),
        w1d=w1d.astype(BF), w2d=w2d.astype(BF), w3d=w3d.astype(BF),
        w4d=w4d.astype(BF), wdd=wdd.astype(BF),
    )


def _emit_mlp_pair(nc, apool, psm, pg, radT_d, w1d, w2d, w3d):
    """MLP layers 1-3 for groups 2*pg (partitions 0-63) and 2*pg+1 (64-127)."""
    rt = apool.tile([128, 512], BF16, tag="radT")
    nc.sync.dma_start(out=rt[0:R], in_=radT_d[0:8, pg * 512:(pg + 1) * 512])
    nc.sync.dma_start(out=rt[64:64 + R],
                      in_=radT_d[8:16, pg * 512:(pg + 1) * 512])

    ps1 = psm.tile([128, 512], F32, tag="mlp")
    nc.tensor.matmul(ps1[0:64], w1d[0:R], rt[0:R], start=True, stop=True,
                     tile_position=(0, 0))
    nc.tensor.matmul(ps1[64:128], w1d[64:64 + R], rt[64:64 + R], start=True,
                     stop=True, tile_position=(64, 64))
    a1 = apool.tile([128, 512], BF16, tag="a1")
    nc.scalar.activation(a1[:], ps1[:], AF.Silu)

    ps2 = psm.tile([128, 512], F32, tag="mlp")
    nc.tensor.matmul(ps2[0:64], w2d[0:64], a1[0:64], start=True, stop=True,
                     tile_position=(0, 0))
    nc.tensor.matmul(ps2[64:128], w2d[64:128], a1[64:128], start=True,
                     stop=True, tile_position=(64, 64))
    a2 = apool.tile([128, 512], BF16, tag="a2")
    nc.scalar.activation(a2[:], ps2[:], AF.Silu)

    ps3 = psm.tile([128, 512], F32, tag="mlp")
    nc.tensor.matmul(ps3[0:64], w3d[0:64], a2[0:64], start=True, stop=True,
                     tile_position=(0, 0))
    nc.tensor.matmul(ps3[64:128], w3d[64:128], a2[64:128], start=True,
                     stop=True, tile_position=(64, 64))
    a3 = apool.tile([128, 512], BF16, tag="a3")
    nc.scalar.activation(a3[:], ps3[:], AF.Silu)
    return a3


def _build(CH):
    SBW = CH * 128
    TOT = NSB * SBW
    NCH = TOT // 128
    NG = TOT // 512
    NPG = NG // 2
    NCALL = TOT // 1024

    nc = bacc.Bacc(None, target_bir_lowering=False, debug=False,
                   dynamic_dma_scratch_size=16384, num_swdge_queues=2)

    nfT_d = nc.dram_tensor("nfT", [C, N_NODES], BF16, kind="ExternalInput")
    wup_d = nc.dram_tensor("wup", [C, C], BF16, kind="ExternalInput")
    w1_d = nc.dram_tensor("w1d", [128, 64], BF16, kind="ExternalInput")
    w2_d = nc.dram_tensor("w2d", [128, 64], BF16, kind="ExternalInput")
    w3_d = nc.dram_tensor("w3d", [128, 64], BF16, kind="ExternalInput")
    w4_d = nc.dram_tensor("w4d", [128, 3 * C], BF16, kind="ExternalInput")
    wdd_d = nc.dram_tensor("wdd", [128, 3, 128], BF16, kind="ExternalInput")
    idx_d = nc.dram_tensor("idx", [128, NCALL * 64], I16, kind="ExternalInput")
    ohY_d = nc.dram_tensor("ohY", [NPAIR, 128, CH * 2 * OHW], BF16,
                           kind="ExternalInput")
    radT_d = nc.dram_tensor("radT", [16, NPG * 512], BF16, kind="ExternalInput")

    h_d = nc.dram_tensor("h", [N_NODES, C], F32)
    out_d = nc.dram_tensor("outp", [NPAIR, 128, OHW], F32, kind="ExternalOutput")

    with tile.TileContext(nc) as tc:
        with tc.tile_pool(name="const", bufs=1) as cpool:
            wup = cpool.tile([C, C], BF16)
            nc.sync.dma_start(out=wup[:], in_=wup_d[:])
            w1d = cpool.tile([128, 64], BF16, tag="w1d")
            nc.sync.dma_start(out=w1d[:], in_=w1_d[:])
            w2d = cpool.tile([128, 64], BF16, tag="w2d")
            nc.sync.dma_start(out=w2d[:], in_=w2_d[:])
            w3d = cpool.tile([128, 64], BF16, tag="w3d")
            nc.sync.dma_start(out=w3d[:], in_=w3_d[:])
            w4d = cpool.tile([128, 3 * C], BF16, tag="w4d")
            nc.sync.dma_start(out=w4d[:], in_=w4_d[:])
            wdd = cpool.tile([128, 3, 128], BF16, tag="wdd")
            nc.sync.dma_start(out=wdd[:], in_=wdd_d[:])
            idxt = cpool.tile([128, NCALL * 64], I16)
            nc.sync.dma_start(out=idxt[:], in_=idx_d[:])

            # all gathered sender features stay resident in SBUF
            sres = cpool.tile([128, NCH, C], F32, tag="sres")

            with tc.tile_pool(name="ohp", bufs=3) as ohp, \
                 tc.tile_pool(name="ap", bufs=4) as apool, \
                 tc.tile_pool(name="tp", bufs=6) as tpool, \
                 tc.tile_pool(name="wr", bufs=2) as wrp, \
                 tc.tile_pool(name="psm", bufs=1, space="PSUM") as psm, \
                 tc.tile_pool(name="psx", bufs=2, space="PSUM") as psx, \
                 tc.tile_pool(name="psa", bufs=2, space="PSUM") as psa, \
                 tc.tile_pool(name="pso", bufs=1, space="PSUM") as pso:

                # ---- phase 1: h = nf @ wup (nfT streamed per batch) ----
                with tc.tile_pool(name="hsb", bufs=3) as hsb:
                    NFULL = N_NODES // 128                    # 156 full tiles
                    for b in range(0, NFULL, 8):
                        nt = min(8, NFULL - b)
                        nft = hsb.tile([C, 8 * 128], BF16, tag="nft")
                        nc.sync.dma_start(
                            out=nft[:, :nt * 128],
                            in_=nfT_d[:, b * 128:(b + nt) * 128])
                        hb = hsb.tile([128, 8, C], F32, tag="hsb")
                        ps = psm.tile([128, 8, C], F32, tag="mlp")
                        for t in range(nt):
                            nc.tensor.matmul(
                                ps[:, t, :],
                                nft[:, t * 128:(t + 1) * 128],
                                wup[:], start=(t == 0), stop=True,
                                skip_group_check=True)
                        nc.vector.tensor_copy(hb[:, :nt, :], ps[:, :nt, :])
                        nc.scalar.dma_start(
                            out=h_d[b * 128:(b + nt) * 128].rearrange(
                                "(t p) c -> p t c", p=128),
                            in_=hb[:, :nt, :])
                    rem = N_NODES - NFULL * 128               # 32 tail rows
                    if rem:
                        nft = hsb.tile([C, 8 * 128], BF16, tag="nft")
                        nc.sync.dma_start(out=nft[:, :rem],
                                          in_=nfT_d[:, NFULL * 128:])
                        ps = psm.tile([128, 8, C], F32, tag="mlp")
                        nc.tensor.matmul(ps[:rem, 0, :], nft[:, :rem],
                                         wup[:], start=True, stop=True)
                        hb = hsb.tile([128, 8, C], F32, tag="hsb")
                        nc.vector.tensor_copy(hb[:rem, 0, :], ps[:rem, 0, :])
                        nc.scalar.dma_start(out=h_d[NFULL * 128:],
                                            in_=hb[:rem, 0, :])

                # gathers are issued just-in-time in the pair loop so their
                # many small SDMA packets don't starve the ohY stream
                issued = [0]

                def issue_gathers(target):
                    while issued[0] < min(target, NCALL):
                        cidx = issued[0]
                        nc.gpsimd.dma_gather(
                            sres[:, cidx * 8:(cidx + 1) * 8, :], h_d[:],
                            idxt[:, cidx * 64:(cidx + 1) * 64],
                            1024, 1024, C, queue_num=cidx % 2)
                        issued[0] += 1

                # ---- main loop over sub-block pairs ----
                # acc layout (double-width, garbage quadrants unused):
                #   acc1 [128, 256]: l0 [0:64] (A=[0:64,0:32], B=[64:128,32:64])
                #                    l1 [64:256] (A=[0:64,64:160], B=[64:,160:])
                #   acc2 [128, 320]: l2 (A=[0:64,0:160], B=[64:128,160:320])
                LW = ((0, 32), (32, 128), (128, OHW))
                next_pg = 0
                a3t = {}
                state = {"next_pg": 0}

                def emit_mixtt(p, j):
                    """mix matmuls + t_all TTs for step (p, j).  Gathers for
                    this step's chunks (plus lead) must be issued BEFORE the
                    sres reads appear in program order."""
                    issue_gathers((2 * (p + 3) * CH) // 8 + 1)
                    tt = tpool.tile([128, 3, 2, C], BF16, tag="t_all")
                    mix = psx.tile([128, 2, 3 * C], F32, tag="mix")
                    for half in range(2):
                        ch = (2 * p + half) * CH + j
                        G, sub = divmod(ch, 4)
                        pg, parity = divmod(G, 2)
                        while state["next_pg"] <= pg and state["next_pg"] < NPG:
                            a3t[state["next_pg"]] = _emit_mlp_pair(
                                nc, apool, psm, state["next_pg"], radT_d,
                                w1d, w2d, w3d)
                            state["next_pg"] += 1
                        p0 = 64 * parity
                        nc.tensor.matmul(
                            mix[:, half, :],
                            a3t[pg][p0:p0 + 64, sub * 128:(sub + 1) * 128],
                            w4d[p0:p0 + 64], start=(half == 0), stop=True,
                            tile_position=(p0, 0),
                            skip_group_check=True)
                        mixv = mix[:, half, :].rearrange(
                            "p (i c) -> p i c", i=3)
                        sv = sres[:, ch, :].unsqueeze(1) \
                            .broadcast_to((128, 3, C))
                        nc.vector.tensor_tensor(tt[:, :, half, :], mixv,
                                                sv, OP.mult)
                    return tt

                steps = [(p, j) for p in range(NPAIR) for j in range(CH)]
                tts = {0: emit_mixtt(*steps[0])}
                for si, (p, j) in enumerate(steps):
                    if j == 0:
                        acc1 = psa.tile([128, 256], F32, tag="acc1")
                        acc2 = psa.tile([128, 320], F32, tag="acc2")
                        ohtp = ohp.tile([128, CH, 2 * OHW], BF16, tag="ohY")
                        nc.sync.dma_start(out=ohtp[:], in_=ohY_d[p])
                    # software pipeline: mix+TT for the NEXT step ahead of
                    # this step's scatter so the PE queue never starves
                    if si + 1 < len(steps):
                        tts[si + 1] = emit_mixtt(*steps[si + 1])
                    tt = tts.pop(si)

                    # 3 scatter matmuls; moving = [ohY_A_l | ohY_B_l]
                    sp = j == CH - 1
                    nc.tensor.matmul(
                        acc1[:, 0:64], tt[:, 0, :, :],
                        ohtp[:, j, 0:64],
                        start=(j == 0), stop=sp, skip_group_check=True)
                    nc.tensor.matmul(
                        acc1[:, 64:256], tt[:, 1, :, :],
                        ohtp[:, j, 64:256],
                        start=False, stop=sp, skip_group_check=True)
                    nc.tensor.matmul(
                        acc2[:], tt[:, 2, :, :],
                        ohtp[:, j, 256:576],
                        start=(j == 0), stop=sp, skip_group_check=True)
                    if j != CH - 1:
                        continue

                    # ---- pair wrap-up: linear_down + output ----
                    # collect valid quadrants into aggs [128, (l0,l1,l2)]
                    aggs = wrp.tile([128, OHW], BF16, tag="aggs")
                    srcs = [(acc1, 0, 0, 32), (acc1, 64, 32, 128),
                            (acc2, 0, 128, OHW)]
                    for l, (accs, a0, c0, c1) in enumerate(srcs):
                        w = c1 - c0
                        nc.scalar.copy(aggs[0:64, c0:c1],
                                       accs[0:64, a0:a0 + w])
                        nc.vector.tensor_copy(aggs[64:128, c0:c1],
                                              accs[64:128, a0 + w:a0 + 2 * w])
                    o = pso.tile([128, OHW], F32, tag="o")
                    for l, (c0, c1) in enumerate(LW):
                        nc.tensor.matmul(
                            o[:, c0:c1], wdd[:, l, :], aggs[:, c0:c1],
                            start=(l == 0), stop=True, skip_group_check=True)
                    osb = wrp.tile([128, OHW], F32, tag="osb")
                    nc.scalar.copy(osb[:], o[:])
                    nc.sync.dma_start(out=out_d[p], in_=osb[:])

    nc.compile()
    return nc


_CACHE = {}


def _get_program(CH):
    if CH not in _CACHE:
        _CACHE[CH] = _build(CH)
    return _CACHE[CH]


def _make_in_maps(prep, sw, node_feats):
    nfT = np.ascontiguousarray(np.asarray(node_feats, np.float32).T).astype(BF)
    maps = []
    for k in range(NCORES):
        maps.append({
            "nfT": nfT, "wup": sw["wup"], "w1d": sw["w1d"], "w2d": sw["w2d"],
            "w3d": sw["w3d"], "w4d": sw["w4d"], "wdd": sw["wdd"],
            "idx": prep["idx"][k], "ohY": prep["ohY"][k],
            "radT": prep["radT"][k],
        })
    return maps


def _assemble(results, node_bin, node_pos):
    out = np.empty((N_NODES, 9 * C), np.float32)
    # A[k, s, d, w] for bin = k*NSB + s
    A = np.stack([r["outp"].reshape(NPAIR, 2, 64, OHW).reshape(NSB, 64, OHW)
                  for r in results])                # [8, NSB, 64, 288]
    A = A.reshape(NCORES * NSB, 64, OHW)            # [NBIN, 64, 288]
    bb = node_bin
    ii = node_pos
    out[:, 0:C] = A[bb, :, ii]                      # [N, 64]
    o1 = np.empty((N_NODES, C, 3), np.float32)
    for m in range(3):
        o1[:, :, m] = A[bb, :, 32 + 32 * m + ii]
    out[:, C:4 * C] = o1.reshape(N_NODES, 3 * C)
    o2 = np.empty((N_NODES, C, 5), np.float32)
    for m in range(5):
        o2[:, :, m] = A[bb, :, 128 + 32 * m + ii]
    out[:, 4 * C:] = o2.reshape(N_NODES, 5 * C)
    return out


def kernel(vectors, node_feats, radial_embedding, senders, receivers,
           w_up, mlp_w1, mlp_w2, mlp_w3, mlp_w4,
           w_down0, w_down1, w_down2):
    prep = _host_prep(vectors, node_feats, radial_embedding, senders, receivers)
    sw = _scaled_weights(w_up, mlp_w1, mlp_w2, mlp_w3, mlp_w4,
                         w_down0, w_down1, w_down2)
    nc = _get_program(prep["CH"])
    in_maps = _make_in_maps(prep, sw, node_feats)
    res = run_bass_kernel_spmd(nc, in_maps, list(range(NCORES)))
    return _assemble(res.results, prep["node_bin"], prep["node_pos"])


# revision 37
# speedup vs baseline: 2.1436x; 1.1596x over previous
"""Trainium2 Bass kernel for the GNN interaction layer (e3nn-style message passing).

Strategy: receiver nodes are packed degree-balanced into 8 cores x 80
sub-blocks of <=32 nodes (greedy LPT), minimizing the padded chunks-per-
sub-block CH (=4 for uniform random graphs, ~2% padding).  The spherical
harmonics are folded into host-precomputed bf16 "Y-scaled one-hot" matrices
ohY[e, (l, m, n32)], streamed from DRAM.  The scatter runs in swapped
orientation on the PE: stationary = per-edge gated features t_l (from the
radial-MLP mix), packed for two sub-blocks side by side in one 128-col
stationary; moving = [ohY_A | ohY_B], so one matmul per irrep covers both
sub-blocks (garbage quadrants accumulate in unused PSUM regions) and the
aggregate lands PSUM-transposed [c, (m,n)] so linear_down is a single
block-diagonal matmul per irrep with no transposes.  Sender features are
dma_gather'ed from a device-computed h table on two SWDGE queues, issued
just-in-time so the small gather packets don't starve the ohY DMA stream;
mix+t for step s+1 are emitted ahead of step s's scatter to keep the PE
queue fed.  All matmuls are bf16.
"""
import math
import numpy as np
import ml_dtypes

from concourse import bacc, mybir, tile
from concourse.bass_utils import run_bass_kernel_spmd

F32 = mybir.dt.float32
BF16 = mybir.dt.bfloat16
I16 = mybir.dt.int16
AF = mybir.ActivationFunctionType
OP = mybir.AluOpType
BF = ml_dtypes.bfloat16

C = 64
R = 8
EPS = 0.5
N_NODES = 20000
N_EDGES = 320000
NCORES = 8
NS = N_NODES // NCORES          # nodes per core (2500)
SBN = 32                        # nodes per sub-block
NSB = 80                        # sub-blocks per core (79 real + 1 pad)
NPAIR = NSB // 2                # sub-block pairs (40)
# ohY column layout per chunk: l0 [0:32], l1 [32:128], l2 [128:288]
OHW = 288


def _spherical(v):
    u = v / np.linalg.norm(v, axis=-1, keepdims=True)
    x, y, z = u[:, 0], u[:, 1], u[:, 2]
    s15 = math.sqrt(15.0)
    y1 = math.sqrt(3.0) * u                                   # [E, 3]
    y2 = np.stack([
        s15 * x * y,
        s15 * y * z,
        0.5 * math.sqrt(5.0) * (3.0 * z * z - 1.0),
        s15 * x * z,
        0.5 * s15 * (x * x - y * y),
    ], axis=-1)                                               # [E, 5]
    return y1.astype(np.float32), y2.astype(np.float32)


def _host_prep(vectors, node_feats, radial, senders, receivers):
    import heapq
    senders = np.asarray(senders)
    receivers = np.asarray(receivers)
    vectors = np.asarray(vectors, np.float32)
    radial = np.asarray(radial, np.float32)

    # degree-balanced assignment of nodes to (core, sub-block) bins so the
    # max per-sub-block edge count (hence CH) is minimal
    NBIN = NCORES * NSB
    deg = np.bincount(receivers, minlength=N_NODES)
    node_bin = np.empty(N_NODES, np.int64)
    node_pos = np.empty(N_NODES, np.int64)
    heap = [(0, 0, b) for b in range(NBIN)]
    heapq.heapify(heap)
    for n in np.argsort(-deg, kind="stable"):
        while True:
            load, cnt, b = heapq.heappop(heap)
            if cnt < SBN:
                break
        node_bin[n] = b
        node_pos[n] = cnt
        heapq.heappush(heap, (load + int(deg[n]), cnt + 1, b))

    ebin = node_bin[receivers]
    n32 = node_pos[receivers]
    gkey = ebin
    order = np.argsort(gkey, kind="stable")
    counts = np.bincount(gkey, minlength=NCORES * NSB)
    CH = max(2, int(math.ceil(counts.max() / 128.0)))
    SBW = CH * 128                                # slots per sub-block
    TOT = NSB * SBW                               # padded slots per core
    NCH = TOT // 128
    NG = TOT // 512
    NPG = NG // 2
    NCALL = TOT // 1024
    assert TOT % 1024 == 0

    # slot index for each edge (in sorted order)
    starts = np.concatenate([[0], np.cumsum(counts)])
    rank = np.arange(len(order)) - starts[gkey[order]]
    g_ord = gkey[order]
    slot = (g_ord % NSB) * SBW + rank             # slot within its core
    core_ord = g_ord // NSB

    y1, y2 = _spherical(vectors)
    y1o, y2o = y1[order], y2[order]
    n32o = n32[order]
    sndo = senders[order].astype(np.int16)
    rado = radial[order]

    snd = np.zeros((NCORES, TOT), np.int16)
    rad = np.zeros((NCORES, TOT, R), np.float32)
    snd[core_ord, slot] = sndo
    rad[core_ord, slot] = rado

    # ohY: [NCORES, TOT, 288] fp32 -> consumption-ordered bf16
    # one row per pair: [128, CH*2*OHW] (j-major, halves side by side)
    ohY_d = np.zeros((NCORES, NPAIR, 128, CH * 2 * OHW), BF)
    for k in range(NCORES):
        m = core_ord == k
        sl = slot[m]
        oh = np.zeros((TOT, OHW), np.float32)
        rows = sl
        oh[rows, n32o[m]] = 1.0
        for mm in range(3):
            oh[rows, 32 + 32 * mm + n32o[m]] = y1o[m, mm]
        for mm in range(5):
            oh[rows, 128 + 32 * mm + n32o[m]] = y2o[m, mm]
        # pair packing: per step j the 576 cols are l-grouped [A_l | B_l]:
        # l0: A32|B32, l1: A96|B96, l2: A160|B160
        ohp = oh.reshape(NPAIR, 2, CH, 128, OHW)
        dst = np.empty((NPAIR, 128, CH, 2 * OHW), np.float32)
        for l, (c0, c1) in enumerate(((0, 32), (32, 128), (128, OHW))):
            w = c1 - c0
            o0 = 2 * c0
            dst[:, :, :, o0:o0 + w] = ohp[:, 0, :, :, c0:c1].transpose(0, 2, 1, 3)
            dst[:, :, :, o0 + w:o0 + 2 * w] = \
                ohp[:, 1, :, :, c0:c1].transpose(0, 2, 1, 3)
        ohY_d[k] = dst.reshape(NPAIR, 128, CH * 2 * OHW).astype(BF)

    # gather idx: wrapped [16, 64] per 1024-slot call, tiled to 128 partitions
    idx = np.zeros((NCORES, 128, NCALL * 64), np.int16)
    for cidx in range(NCALL):
        blk = snd[:, cidx * 1024:(cidx + 1) * 1024]          # [NCORES, 1024]
        wrapped = blk.reshape(NCORES, 64, 16).transpose(0, 2, 1)
        idx[:, :, cidx * 64:(cidx + 1) * 64] = np.tile(wrapped, (1, 8, 1))

    # radial, transposed, packed per MLP pair: rows 0:8 even group, 8:16 odd
    radT = np.zeros((NCORES, 16, NPG * 512), BF)
    radt = rad.transpose(0, 2, 1)                            # [NCORES, R, TOT]
    for pg in range(NPG):
        radT[:, 0:8, pg * 512:(pg + 1) * 512] = \
            radt[:, :, (2 * pg) * 512:(2 * pg + 1) * 512]
        radT[:, 8:16, pg * 512:(pg + 1) * 512] = \
            radt[:, :, (2 * pg + 1) * 512:(2 * pg + 2) * 512]

    return dict(CH=CH, TOT=TOT, NCH=NCH, NG=NG, NPG=NPG, NCALL=NCALL,
                idx=idx, ohY=ohY_d, radT=radT,
                node_bin=node_bin, node_pos=node_pos)


def _scaled_weights(w_up, w1, w2, w3, w4, wd0, wd1, wd2):
    inv_sqrt_c = 1.0 / math.sqrt(C)
    w1s = (np.asarray(w1) / math.sqrt(R)).astype(np.float32)
    w2s = (np.asarray(w2) / math.sqrt(64.0)).astype(np.float32)
    w3s = (np.asarray(w3) / math.sqrt(64.0)).astype(np.float32)
    w4s = (np.asarray(w4) * (1.0 / math.sqrt(64.0)) * (1.0 / C)).astype(np.float32)
    w1d = np.zeros((128, 64), np.float32)
    w1d[0:R] = w1s
    w1d[64:64 + R] = w1s
    w2d = np.concatenate([w2s, w2s], axis=0)
    w3d = np.concatenate([w3s, w3s], axis=0)
    w4d = np.concatenate([w4s, w4s], axis=0)
    # block-diagonal per irrep: one matmul handles both halves
    wdd = np.zeros((128, 3, 128), np.float32)
    for i, wd in enumerate((wd0, wd1, wd2)):
        s = np.asarray(wd) * EPS * inv_sqrt_c
        wdd[0:64, i, 0:64] = s
        wdd[64:128, i, 64:128] = s
    return dict(
        wup=(np.asarray(w_up) * inv_sqrt_c).astype(BF),
        w1d=w1d.astype(BF), w2d=w2d.astype(BF), w3d=w3d.astype(BF),
        w4d=w4d.astype(BF), wdd=wdd.astype(BF),
    )


def _emit_mlp_pair(nc, apool, psm, pg, radT_d, w1d, w2d, w3d):
    """MLP layers 1-3 for groups 2*pg (partitions 0-63) and 2*pg+1 (64-127)."""
    rt = apool.tile([128, 512], BF16, tag="radT")
    nc.sync.dma_start(out=rt[0:R], in_=radT_d[0:8, pg * 512:(pg + 1) * 512])
    nc.sync.dma_start(out=rt[64:64 + R],
                      in_=radT_d[8:16, pg * 512:(pg + 1) * 512])

    ps1 = psm.tile([128, 512], F32, tag="mlp")
    nc.tensor.matmul(ps1[0:64], w1d[0:R], rt[0:R], start=True, stop=True,
                     tile_position=(0, 0))
    nc.tensor.matmul(ps1[64:128], w1d[64:64 + R], rt[64:64 + R], start=True,
                     stop=True, tile_position=(64, 64))
    a1 = apool.tile([128, 512], BF16, tag="a1")
    nc.scalar.activation(a1[:], ps1[:], AF.Silu)

    ps2 = psm.tile([128, 512], F32, tag="mlp")
    nc.tensor.matmul(ps2[0:64], w2d[0:64], a1[0:64], start=True, stop=True,
                     tile_position=(0, 0))
    nc.tensor.matmul(ps2[64:128], w2d[64:128], a1[64:128], start=True,
                     stop=True, tile_position=(64, 64))
    a2 = apool.tile([128, 512], BF16, tag="a2")
    nc.scalar.activation(a2[:], ps2[:], AF.Silu)

    ps3 = psm.tile([128, 512], F32, tag="mlp")
    nc.tensor.matmul(ps3[0:64], w3d[0:64], a2[0:64], start=True, stop=True,
                     tile_position=(0, 0))
    nc.tensor.matmul(ps3[64:128], w3d[64:128], a2[64:128], start=True,
                     stop=True, tile_position=(64, 64))
    a3 = apool.tile([128, 512], BF16, tag="a3")
    nc.scalar.activation(a3[:], ps3[:], AF.Silu)
    return a3


def _build(CH):
    SBW = CH * 128
    TOT = NSB * SBW
    NCH = TOT // 128
    NG = TOT // 512
    NPG = NG // 2
    NCALL = TOT // 1024

    nc = bacc.Bacc(None, target_bir_lowering=False, debug=False,
                   dynamic_dma_scratch_size=16384, num_swdge_queues=2)

    nfT_d = nc.dram_tensor("nfT", [C, N_NODES], BF16, kind="ExternalInput")
    wup_d = nc.dram_tensor("wup", [C, C], BF16, kind="ExternalInput")
    w1_d = nc.dram_tensor("w1d", [128, 64], BF16, kind="ExternalInput")
    w2_d = nc.dram_tensor("w2d", [128, 64], BF16, kind="ExternalInput")
    w3_d = nc.dram_tensor("w3d", [128, 64], BF16, kind="ExternalInput")
    w4_d = nc.dram_tensor("w4d", [128, 3 * C], BF16, kind="ExternalInput")
    wdd_d = nc.dram_tensor("wdd", [128, 3, 128], BF16, kind="ExternalInput")
    idx_d = nc.dram_tensor("idx", [128, NCALL * 64], I16, kind="ExternalInput")
    ohY_d = nc.dram_tensor("ohY", [NPAIR, 128, CH * 2 * OHW], BF16,
                           kind="ExternalInput")
    radT_d = nc.dram_tensor("radT", [16, NPG * 512], BF16, kind="ExternalInput")

    h_d = nc.dram_tensor("h", [N_NODES, C], F32)
    out_d = nc.dram_tensor("outp", [NPAIR, 128, OHW], F32, kind="ExternalOutput")

    with tile.TileContext(nc) as tc:
        with tc.tile_pool(name="const", bufs=1) as cpool:
            wup = cpool.tile([C, C], BF16)
            nc.sync.dma_start(out=wup[:], in_=wup_d[:])
            w1d = cpool.tile([128, 64], BF16, tag="w1d")
            nc.sync.dma_start(out=w1d[:], in_=w1_d[:])
            w2d = cpool.tile([128, 64], BF16, tag="w2d")
            nc.sync.dma_start(out=w2d[:], in_=w2_d[:])
            w3d = cpool.tile([128, 64], BF16, tag="w3d")
            nc.sync.dma_start(out=w3d[:], in_=w3_d[:])
            w4d = cpool.tile([128, 3 * C], BF16, tag="w4d")
            nc.sync.dma_start(out=w4d[:], in_=w4_d[:])
            wdd = cpool.tile([128, 3, 128], BF16, tag="wdd")
            nc.sync.dma_start(out=wdd[:], in_=wdd_d[:])
            idxt = cpool.tile([128, NCALL * 64], I16)
            nc.sync.dma_start(out=idxt[:], in_=idx_d[:])

            # all gathered sender features stay resident in SBUF
            sres = cpool.tile([128, NCH, C], F32, tag="sres")

            with tc.tile_pool(name="ohp", bufs=3) as ohp, \
                 tc.tile_pool(name="ap", bufs=5) as apool, \
                 tc.tile_pool(name="tp", bufs=6) as tpool, \
                 tc.tile_pool(name="wr", bufs=2) as wrp, \
                 tc.tile_pool(name="psm", bufs=1, space="PSUM") as psm, \
                 tc.tile_pool(name="psx", bufs=2, space="PSUM") as psx, \
                 tc.tile_pool(name="psa", bufs=2, space="PSUM") as psa, \
                 tc.tile_pool(name="pso", bufs=1, space="PSUM") as pso:

                # ---- phase 1: h = nf @ wup (nfT streamed per batch) ----
                with tc.tile_pool(name="hsb", bufs=3) as hsb:
                    NFULL = N_NODES // 128                    # 156 full tiles
                    for b in range(0, NFULL, 8):
                        nt = min(8, NFULL - b)
                        nft = hsb.tile([C, 8 * 128], BF16, tag="nft")
                        nc.sync.dma_start(
                            out=nft[:, :nt * 128],
                            in_=nfT_d[:, b * 128:(b + nt) * 128])
                        hb = hsb.tile([128, 8, C], F32, tag="hsb")
                        ps = psm.tile([128, 8, C], F32, tag="mlp")
                        for t in range(nt):
                            nc.tensor.matmul(
                                ps[:, t, :],
                                nft[:, t * 128:(t + 1) * 128],
                                wup[:], start=(t == 0), stop=True,
                                skip_group_check=True)
                        nc.vector.tensor_copy(hb[:, :nt, :], ps[:, :nt, :])
                        nc.scalar.dma_start(
                            out=h_d[b * 128:(b + nt) * 128].rearrange(
                                "(t p) c -> p t c", p=128),
                            in_=hb[:, :nt, :])
                    rem = N_NODES - NFULL * 128               # 32 tail rows
                    if rem:
                        nft = hsb.tile([C, 8 * 128], BF16, tag="nft")
                        nc.sync.dma_start(out=nft[:, :rem],
                                          in_=nfT_d[:, NFULL * 128:])
                        ps = psm.tile([128, 8, C], F32, tag="mlp")
                        nc.tensor.matmul(ps[:rem, 0, :], nft[:, :rem],
                                         wup[:], start=True, stop=True)
                        hb = hsb.tile([128, 8, C], F32, tag="hsb")
                        nc.vector.tensor_copy(hb[:rem, 0, :], ps[:rem, 0, :])
                        nc.scalar.dma_start(out=h_d[NFULL * 128:],
                                            in_=hb[:rem, 0, :])

                # gathers are issued just-in-time in the pair loop so their
                # many small SDMA packets don't starve the ohY stream
                issued = [0]

                def issue_gathers(target):
                    while issued[0] < min(target, NCALL):
                        cidx = issued[0]
                        nc.gpsimd.dma_gather(
                            sres[:, cidx * 8:(cidx + 1) * 8, :], h_d[:],
                            idxt[:, cidx * 64:(cidx + 1) * 64],
                            1024, 1024, C, queue_num=cidx % 2)
                        issued[0] += 1

                # ---- main loop over sub-block pairs ----
                # acc layout (double-width, garbage quadrants unused):
                #   acc1 [128, 256]: l0 [0:64] (A=[0:64,0:32], B=[64:128,32:64])
                #                    l1 [64:256] (A=[0:64,64:160], B=[64:,160:])
                #   acc2 [128, 320]: l2 (A=[0:64,0:160], B=[64:128,160:320])
                LW = ((0, 32), (32, 128), (128, OHW))
                next_pg = 0
                a3t = {}
                state = {"next_pg": 0}

                def emit_mixtt(p, j):
                    """mix matmuls + t_all TTs for step (p, j).  Gathers for
                    this step's chunks (plus lead) must be issued BEFORE the
                    sres reads appear in program order."""
                    issue_gathers((2 * (p + 3) * CH) // 8 + 1)
                    tt = tpool.tile([128, 3, 2, C], BF16, tag="t_all")
                    mix = psx.tile([128, 2, 3 * C], F32, tag="mix")
                    for half in range(2):
                        ch = (2 * p + half) * CH + j
                        G, sub = divmod(ch, 4)
                        pg, parity = divmod(G, 2)
                        # emit MLP one pair ahead so mix never waits on silu
                        while state["next_pg"] <= pg + 1 and state["next_pg"] < NPG:
                            a3t[state["next_pg"]] = _emit_mlp_pair(
                                nc, apool, psm, state["next_pg"], radT_d,
                                w1d, w2d, w3d)
                            state["next_pg"] += 1
                        p0 = 64 * parity
                        nc.tensor.matmul(
                            mix[:, half, :],
                            a3t[pg][p0:p0 + 64, sub * 128:(sub + 1) * 128],
                            w4d[p0:p0 + 64], start=(half == 0), stop=True,
                            tile_position=(p0, 0),
                            skip_group_check=True)
                        mixv = mix[:, half, :].rearrange(
                            "p (i c) -> p i c", i=3)
                        sv = sres[:, ch, :].unsqueeze(1) \
                            .broadcast_to((128, 3, C))
                        nc.vector.tensor_tensor(tt[:, :, half, :], mixv,
                                                sv, OP.mult)
                    return tt

                steps = [(p, j) for p in range(NPAIR) for j in range(CH)]
                tts = {0: emit_mixtt(*steps[0])}
                for si, (p, j) in enumerate(steps):
                    if j == 0:
                        acc1 = psa.tile([128, 256], F32, tag="acc1")
                        acc2 = psa.tile([128, 320], F32, tag="acc2")
                        ohtp = ohp.tile([128, CH, 2 * OHW], BF16, tag="ohY")
                        nc.sync.dma_start(out=ohtp[:], in_=ohY_d[p])
                    # software pipeline: mix+TT for the NEXT step ahead of
                    # this step's scatter so the PE queue never starves
                    if si + 1 < len(steps):
                        tts[si + 1] = emit_mixtt(*steps[si + 1])
                    tt = tts.pop(si)

                    # 3 scatter matmuls; moving = [ohY_A_l | ohY_B_l]
                    sp = j == CH - 1
                    nc.tensor.matmul(
                        acc1[:, 0:64], tt[:, 0, :, :],
                        ohtp[:, j, 0:64],
                        start=(j == 0), stop=sp, skip_group_check=True)
                    nc.tensor.matmul(
                        acc1[:, 64:256], tt[:, 1, :, :],
                        ohtp[:, j, 64:256],
                        start=False, stop=sp, skip_group_check=True)
                    nc.tensor.matmul(
                        acc2[:], tt[:, 2, :, :],
                        ohtp[:, j, 256:576],
                        start=(j == 0), stop=sp, skip_group_check=True)
                    if j != CH - 1:
                        continue

                    # ---- pair wrap-up: linear_down + output ----
                    # collect valid quadrants into aggs [128, (l0,l1,l2)]
                    aggs = wrp.tile([128, OHW], BF16, tag="aggs")
                    srcs = [(acc1, 0, 0, 32), (acc1, 64, 32, 128),
                            (acc2, 0, 128, OHW)]
                    for l, (accs, a0, c0, c1) in enumerate(srcs):
                        w = c1 - c0
                        nc.scalar.copy(aggs[0:64, c0:c1],
                                       accs[0:64, a0:a0 + w])
                        nc.vector.tensor_copy(aggs[64:128, c0:c1],
                                              accs[64:128, a0 + w:a0 + 2 * w])
                    o = pso.tile([128, OHW], F32, tag="o")
                    for l, (c0, c1) in enumerate(LW):
                        nc.tensor.matmul(
                            o[:, c0:c1], wdd[:, l, :], aggs[:, c0:c1],
                            start=(l == 0), stop=True, skip_group_check=True)
                    osb = wrp.tile([128, OHW], F32, tag="osb")
                    nc.scalar.copy(osb[:], o[:])
                    nc.sync.dma_start(out=out_d[p], in_=osb[:])

    nc.compile()
    return nc


_CACHE = {}


def _get_program(CH):
    if CH not in _CACHE:
        _CACHE[CH] = _build(CH)
    return _CACHE[CH]


def _make_in_maps(prep, sw, node_feats):
    nfT = np.ascontiguousarray(np.asarray(node_feats, np.float32).T).astype(BF)
    maps = []
    for k in range(NCORES):
        maps.append({
            "nfT": nfT, "wup": sw["wup"], "w1d": sw["w1d"], "w2d": sw["w2d"],
            "w3d": sw["w3d"], "w4d": sw["w4d"], "wdd": sw["wdd"],
            "idx": prep["idx"][k], "ohY": prep["ohY"][k],
            "radT": prep["radT"][k],
        })
    return maps


def _assemble(results, node_bin, node_pos):
    out = np.empty((N_NODES, 9 * C), np.float32)
    # A[k, s, d, w] for bin = k*NSB + s
    A = np.stack([r["outp"].reshape(NPAIR, 2, 64, OHW).reshape(NSB, 64, OHW)
                  for r in results])                # [8, NSB, 64, 288]
    A = A.reshape(NCORES * NSB, 64, OHW)            # [NBIN, 64, 288]
    bb = node_bin
    ii = node_pos
    out[:, 0:C] = A[bb, :, ii]                      # [N, 64]
    o1 = np.empty((N_NODES, C, 3), np.float32)
    for m in range(3):
        o1[:, :, m] = A[bb, :, 32 + 32 * m + ii]
    out[:, C:4 * C] = o1.reshape(N_NODES, 3 * C)
    o2 = np.empty((N_NODES, C, 5), np.float32)
    for m in range(5):
        o2[:, :, m] = A[bb, :, 128 + 32 * m + ii]
    out[:, 4 * C:] = o2.reshape(N_NODES, 5 * C)
    return out


def kernel(vectors, node_feats, radial_embedding, senders, receivers,
           w_up, mlp_w1, mlp_w2, mlp_w3, mlp_w4,
           w_down0, w_down1, w_down2):
    prep = _host_prep(vectors, node_feats, radial_embedding, senders, receivers)
    sw = _scaled_weights(w_up, mlp_w1, mlp_w2, mlp_w3, mlp_w4,
                         w_down0, w_down1, w_down2)
    nc = _get_program(prep["CH"])
    in_maps = _make_in_maps(prep, sw, node_feats)
    res = run_bass_kernel_spmd(nc, in_maps, list(range(NCORES)))
    return _assemble(res.results, prep["node_bin"], prep["node_pos"])
